# revision 26
# baseline (speedup 1.0000x reference)
"""Trainium2 Bass kernel for the signature-kernel (Goursat PDE) problem.

Full inputs: xs (32, 64, 16) f32, ys (32, 64, 16) f32.
Output: (32, 32) f32 signature-kernel Gram matrix.

Strategy (8 NeuronCores, SPMD, no collectives):
  - Shard batch_x across cores: core c owns a in {4c..4c+3} -> 4*32 = 128
    (x, y) pairs, one pair per SBUF partition.
  - Coarse increments inc[a,b,I,J] = sum_d Dxs[a,I,d] Dys[b,J,d] / 4 are
    computed on-device with PE matmuls from DENSE hi/lo bf16 splits: per
    coarse row J and per a-group g, out partitions 32g..32g+32 contract
    lhsT = DysT[d, J, b] (16x32) against rhs = Dxs_g[d, I] (16x63); three
    accumulating matmuls (hi*hi + hi*lo + lo*hi) give fp32-level accuracy.
  - The Goursat PDE recurrence K[i+1,j+1] = c1*(K[i+1,j] + K[i,j+1])
    - c2*K[i,j] with the dyadically-refined (2x-duplicated) coefficients
    c1 = 1 + v/2 + v^2/12, c2 = 1 - v^2/12 is solved with ONE custom DVE
    instruction per ROW PAIR (63 instructions, get_dbl_op): dyadic
    refinement duplicates rows, so rows 2I and 2I+1 share coefficients
    and a hand-written 4-uop program advances BOTH in one pass at 1
    stream element/cycle -- u' = c1*(u + K0[j+1] + gamma*K0[j]),
    v' = c1*(v + u' + gamma*u) -- emitting only v (the intermediate row
    never materializes). gamma = -c2/c1 = -1 + v/2 - v^2/12 + O(v^4).
    The v accumulator crosses elements through slice 7's operand flop
    (alu_out_a_enable on the O element, NEXT_ALU_OUT_A read on C).
    Coefficients stream as [gamma_j, c1_j, c1_j] per fine column (379
    elements, ~553 ns per double row), produced per chunk by two Part-I
    custom poly ops (PSUM vf -> interleaved coarse pairs) and expanded
    by the otherwise-idle Act engine.
  - The final column K[*, 126] is compacted with a 32x32 StreamTranspose
    so the output DMA is 4 descriptors instead of 128.
"""

import os
import sys

import numpy as np

for _p in ("/opt/trn_rl_repo", "/root/.axon_site", "/root/.axon_site/_ro/trn_rl_repo",
           "/root/.axon_site/_ro/pypackages"):
    if os.path.isdir(_p) and _p not in sys.path:
        sys.path.append(_p)

_STATE: dict = {}
_OP2 = None

_OPS_CACHE: dict = {}


def _register_hand_op(name, make_uops, rd1, ref, spec_body):
    """Register a hand-written custom DVE op; returns the DveOp."""
    import concourse.dve_ops as dve_ops
    from concourse.dve_ops import DveOp, OPS
    from concourse.dve_spec import Spec
    from concourse.dve_uop import DveOpSpec

    for op in OPS:
        if op.name == name:
            return op

    class _H(DveOp):
        def compile(self, ver):
            spec = DveOpSpec(
                name=self.name,
                opcode=dve_ops.get_dve_sub_opcode(self.name),
                uops=make_uops(),
                rd1_en=rd1,
            )
            spec.validate(ver)
            return spec

    op = _H(name=name, spec=Spec(body=spec_body, reference=ref),
            subdim=False, uops_sha={})
    dve_ops._SUB_OPCODE_FOR_NAME[op.name] = 1 + len(OPS)
    OPS.append(op)
    dve_ops.CUSTOM_DVE_SPECS[op.name] = op.spec
    return op


def _register_spec_op(name, body, ref):
    """Register a Part-I Spec op (auto-lowered), bypassing the sha pin."""
    import concourse.dve_ops as dve_ops
    from concourse.dve_ops import DveOp, OPS
    from concourse.dve_spec import Spec, lower, _has_src1
    from concourse.dve_uop import DveOpSpec

    for op in OPS:
        if op.name == name:
            return op

    class _S(DveOp):
        def compile(self, ver):
            spec = DveOpSpec(
                name=self.name,
                opcode=dve_ops.get_dve_sub_opcode(self.name),
                uops=lower(self.spec, ver=ver),
                rd1_en=_has_src1(self.spec),
            )
            spec.validate(ver)
            return spec

    op = _S(name=name, spec=Spec(body=body, reference=ref),
            subdim=False, uops_sha={})
    dve_ops._SUB_OPCODE_FOR_NAME[op.name] = 1 + len(OPS)
    OPS.append(op)
    dve_ops.CUSTOM_DVE_SPECS[op.name] = op.spec
    return op


def get_poly_ops():
    """c1 = 1 + v*s0 + v^2*s1 ; gamma = -1 + v*s0 - v^2*s1 (= -c2/c1 +O(v^4))."""
    from concourse.dve_spec import Src0, C0, C1, One, sq

    c1 = _register_spec_op(
        "C1_POLY_ANT",
        Src0 * C0 + sq(Src0) * C1 + One,
        lambda in0, in1, s0, s1, imm2: (
            1.0 + in0.astype("float64") * s0 + in0.astype("float64") ** 2 * s1
        ).astype("float32"),
    )
    gm = _register_spec_op(
        "GAMMA_POLY_ANT",
        Src0 * C0 - sq(Src0) * C1 - One,
        lambda in0, in1, s0, s1, imm2: (
            in0.astype("float64") * s0 - in0.astype("float64") ** 2 * s1 - 1.0
        ).astype("float32"),
    )
    return c1, gm


def get_dbl_op():
    """Two PDE rows per instruction; see exp_custom3.py for the derivation.

    Stream (3 elements per fine column j): SRC_0 = [gamma_j, c1_j, c1_j],
    SRC_1 = K0[j+1] (x3). Per pair: u' = c1*(u + K0[j+1] + gamma*K0[j]),
    v' = c1*(v + u' + gamma*u); only v' is emitted (the intermediate row u
    never materializes). v is handed across elements via slice 7's a-flop
    (alu_out_a_enable on O, NEXT_ALU_OUT_A read on C at slice 6).
    """
    from concourse.dve_uop import (
        UopConfig, AluOp, AluInp, InpSel, OutSel, OutPath, Trigger, DelayInp,
    )
    import numpy as np

    def mk_init():
        u = UopConfig()
        u.enable_input(InpSel.ONE_F32, 1)
        u.repeat_count = 1
        u.trigger = (Trigger.COUNT, Trigger.NONE, Trigger.NONE)
        u.next_uop = (1, 0, 0)
        dp = u.datapath_config
        dp[0].enable_alu(AluOp.BYPASS, AluInp.PREV_DELAY_0)
        for b in range(7):
            dp[b].pass_through_delay(0)
        dp[5].enable_alu(AluOp.BYPASS, AluInp.PREV_DELAY_0)
        dp[7].enable_alu(AluOp.BYPASS, AluInp.PREV_DELAY_0)
        dp[7].alu_out_a_enable = 1
        return u

    def mk_O():
        u = UopConfig()
        u.enable_input(InpSel.SRC_0, 1)
        u.enable_input(InpSel.SRC_1, 2)
        u.require_inp0 = u.require_inp1 = 1
        u.repeat_count = 1
        u.trigger = (Trigger.SRC_TENSOR_DONE, Trigger.COUNT, Trigger.NONE)
        u.next_uop = (0, 2, 0)
        dp = u.datapath_config
        dp[0].enable_alu(AluOp.MULTIPLY, AluInp.PREV_DELAY_0, AluInp.CURR_ALU_OUT)
        dp[0].pass_through_delay(0, 1)
        dp[1].enable_alu(AluOp.ADD, AluInp.PREV_ALU_OUT, AluInp.PREV_DELAY_1)
        dp[1].pass_through_delay(0)
        dp[2].enable_delay_from_src(DelayInp.PREV_ALU_OUT, 1)
        dp[2].pass_through_delay(0)
        dp[3].enable_alu(AluOp.BYPASS, AluInp.PREV_DELAY_0)  # flop3 := gamma_j
        dp[3].pass_through_delay(0, 1)
        dp[4].pass_through_delay(0, 1)
        dp[5].enable_alu(AluOp.ADD, AluInp.PREV_DELAY_1, AluInp.CURR_ALU_OUT)
        dp[5].enable_delay_from_src(DelayInp.CURR_ALU_OUT, 2)
        dp[5].pass_through_delay(0)
        dp[6].enable_alu(AluOp.MULTIPLY, AluInp.PREV_DELAY_0, AluInp.PREV_DELAY_2)
        dp[7].enable_alu(AluOp.BYPASS, AluInp.CURR_ALU_OUT, AluInp.CURR_ALU_OUT)
        dp[7].alu_out_a_enable = 1
        return u

    def mk_E():
        u = UopConfig()
        u.enable_input(InpSel.SRC_0, 1)
        u.enable_input(InpSel.SRC_1, 2)
        u.require_inp0 = u.require_inp1 = 1
        u.repeat_count = 1
        u.trigger = (Trigger.SRC_TENSOR_DONE, Trigger.COUNT, Trigger.NONE)
        u.next_uop = (0, 3, 0)
        dp = u.datapath_config
        dp[0].enable_alu(AluOp.BYPASS, AluInp.PREV_DELAY_1)
        for b in range(3):
            dp[b].pass_through_delay(0)
        # s3: c1 = v - gamma (gamma parked in flop3 by O)
        dp[3].enable_alu(AluOp.SUBTRACT, AluInp.PREV_DELAY_0, AluInp.CURR_ALU_OUT)
        dp[4].pass_through_alu()  # carry c1
        dp[5].enable_alu(AluOp.MULTIPLY, AluInp.CURR_ALU_OUT, AluInp.PREV_ALU_OUT)
        dp[6].enable_alu(AluOp.ADD, AluInp.CURR_ALU_OUT, AluInp.PREV_ALU_OUT)
        return u

    def mk_C():
        u = UopConfig()
        u.enable_input(InpSel.SRC_0, 1)
        u.enable_input(InpSel.SRC_1, 2)
        u.require_inp0 = u.require_inp1 = 1
        u.repeat_count = 1
        u.trigger = (Trigger.SRC_TENSOR_DONE, Trigger.COUNT, Trigger.NONE)
        u.next_uop = (0, 1, 0)
        dp = u.datapath_config
        for b in range(6):
            dp[b].pass_through_delay(0)
        # s3: pick up c1 (flop3, written by E one cycle earlier) into lane1
        dp[3].enable_delay_from_src(DelayInp.CURR_ALU_OUT, 1)
        dp[4].pass_through_delay(1)
        dp[5].pass_through_delay(1)
        dp[6].enable_alu(AluOp.ADD, AluInp.CURR_ALU_OUT, AluInp.NEXT_ALU_OUT_A)
        dp[6].pass_through_delay(0, 1)
        dp[7].enable_alu(AluOp.MULTIPLY, AluInp.PREV_ALU_OUT, AluInp.PREV_DELAY_1)
        u.enable_output(OutSel.ALU_OUT, OutPath.WR0_LO)
        return u

    def ref(in0, in1, s0, s1, imm2):
        p = in0.shape[0]
        n = in0.shape[-1] // 3
        cc = in0.reshape(p, n, 3).astype(np.float64)
        kk = in1.reshape(p, n, 3)[:, :, 0].astype(np.float64)
        gam = cc[:, :, 0]
        c1 = cc[:, :, 1] - gam  # slot 1 carries v; c1 = v - gamma
        out = np.empty((p, n), np.float64)
        u = np.ones(p); v = np.ones(p); kprev = np.ones(p)
        for j in range(n):
            un = c1[:, j] * (u + kk[:, j] + gam[:, j] * kprev)
            v = c1[:, j] * (v + un + gam[:, j] * u)
            u, kprev = un, kk[:, j]
            out[:, j] = v
        return out.astype(np.float32)

    from concourse.dve_spec import Src0, Src1

    return _register_hand_op(
        "DBL_PAIR_SCAN_ANT", lambda: [mk_init(), mk_O(), mk_E(), mk_C()],
        True, ref, Src0 * Src1,
    )




def get_vf_scan_op():
    """Custom DVE op VF_PAIR_SCAN_ANT (hand-written 3-uop program).

    Reads the coarse vf value v_j (duplicated 4x via a stride-0 AP) and the
    previous K row (K[j+1] duplicated 2x), computes c1/gamma inline:
        c1 = 1 + v/2 + v^2/12,  gamma = -1 + v/2 - v^2/12  (= -c2/c1 + O(v^4))
        acc_j = c1_j * (acc_{j-1} + K[j+1] + gamma_j * K[j])
    and emits acc_j (= K_new[j+1]) on every second stream element, at one
    stream element per cycle. s0 = 0.5 (CONST_0), s1 = 1/12 (CONST_1).
    """
    global _OP2
    if _OP2 is not None:
        return _OP2
    import concourse.dve_ops as dve_ops
    from concourse.dve_ops import DveOp, OPS
    from concourse.dve_spec import Spec, Src0, Src1, C0, C1
    from concourse.dve_uop import (
        DveOpSpec,
        UopConfig,
        AluOp,
        AluInp,
        InpSel,
        OutSel,
        OutPath,
        Trigger,
        DelayInp,
    )

    for op in OPS:
        if op.name == "VF_PAIR_SCAN_ANT":
            _OP2 = op
            return op

    def _inputs(u):
        u.enable_input(InpSel.SRC_0, 0)    # v -> blk0 PREV_ALU_OUT
        u.enable_input(InpSel.SRC_0, 1)    # v -> lane 0
        u.enable_input(InpSel.SRC_1, 2)    # K[j+1] -> lane 1
        u.enable_input(InpSel.CONST_0, 3)  # 0.5 -> lane 2
        u.enable_input(InpSel.CONST_1, 4)  # 1/12 -> lane 3
        u.enable_input(InpSel.ONE_F32, 5)  # 1.0 -> lane 4
        u.require_inp0 = u.require_inp1 = 1
        u.repeat_count = 1
        return u

    def _mk_init():
        u = UopConfig()
        u.enable_input(InpSel.ONE_F32, 1)  # lane 0
        u.repeat_count = 1
        u.trigger = (Trigger.COUNT, Trigger.NONE, Trigger.NONE)
        u.next_uop = (1, 0, 0)
        dp = u.datapath_config
        dp[0].pass_through_delay(0)
        dp[1].enable_alu(AluOp.BYPASS, AluInp.PREV_DELAY_0)  # flop1 := 1.0 (K[0])
        for b in (1, 2, 3, 4, 5, 6):
            dp[b].pass_through_delay(0)
        dp[7].enable_alu(AluOp.BYPASS, AluInp.PREV_DELAY_0)  # flop7 := 1.0 (acc)
        return u

    def _mk_odd():
        # even stream elements (2j): compute gamma_j, p, m, u
        u = _inputs(UopConfig())
        u.trigger = (Trigger.SRC_TENSOR_DONE, Trigger.COUNT, Trigger.NONE)
        u.next_uop = (0, 2, 0)
        dp = u.datapath_config
        dp[0].enable_alu(AluOp.MULTIPLY, AluInp.PREV_ALU_OUT, AluInp.PREV_DELAY_2)
        dp[0].pass_through_delay(0, 1, 3, 4)
        dp[1].enable_alu(AluOp.MULTIPLY, AluInp.PREV_DELAY_0, AluInp.PREV_DELAY_0)
        dp[1].enable_delay_from_src(DelayInp.CURR_ALU_OUT, 5)  # K[j] handoff read
        dp[1].enable_delay_from_src(DelayInp.PREV_ALU_OUT, 0)  # a = v/2
        dp[1].pass_through_delay(1, 3, 4)
        dp[2].enable_alu(AluOp.MULTIPLY, AluInp.PREV_ALU_OUT, AluInp.PREV_DELAY_3)
        dp[2].pass_through_delay(0, 1, 4, 5)
        dp[3].enable_alu(AluOp.SUBTRACT, AluInp.PREV_DELAY_0, AluInp.PREV_DELAY_4)
        dp[3].enable_delay_from_src(DelayInp.PREV_ALU_OUT, 0)  # b = vv/12
        dp[3].pass_through_delay(1, 5)
        dp[4].enable_alu(AluOp.SUBTRACT, AluInp.PREV_ALU_OUT, AluInp.PREV_DELAY_0)
        dp[4].pass_through_delay(1, 5)
        dp[5].enable_alu(AluOp.MULTIPLY, AluInp.PREV_ALU_OUT, AluInp.PREV_DELAY_5)
        dp[5].pass_through_delay(1)
        dp[6].enable_alu(AluOp.ADD, AluInp.PREV_ALU_OUT, AluInp.PREV_DELAY_1)
        dp[7].enable_alu(AluOp.ADD, AluInp.PREV_ALU_OUT, AluInp.CURR_ALU_OUT)
        return u

    def _mk_even():
        # odd stream elements (2j+1): c1_j, K handoff, acc = u * c1; emit
        u = _inputs(UopConfig())
        u.trigger = (Trigger.SRC_TENSOR_DONE, Trigger.COUNT, Trigger.NONE)
        u.next_uop = (0, 1, 0)
        dp = u.datapath_config
        dp[0].enable_alu(AluOp.MULTIPLY, AluInp.PREV_ALU_OUT, AluInp.PREV_DELAY_0)
        dp[0].pass_through_delay(0, 1, 2, 3, 4)
        dp[1].enable_alu(AluOp.BYPASS, AluInp.PREV_DELAY_1)   # flop1 := K[j+1]
        dp[1].enable_delay_from_src(DelayInp.PREV_ALU_OUT, 5)  # vv
        dp[1].pass_through_delay(0, 2, 3, 4)
        dp[2].enable_alu(AluOp.MULTIPLY, AluInp.PREV_DELAY_0, AluInp.PREV_DELAY_2)
        dp[2].pass_through_delay(3, 4, 5)
        dp[3].enable_alu(AluOp.MULTIPLY, AluInp.PREV_DELAY_5, AluInp.PREV_DELAY_3)
        dp[3].enable_delay_from_src(DelayInp.PREV_ALU_OUT, 0)  # a = v/2
        dp[3].pass_through_delay(4)
        dp[4].enable_alu(AluOp.ADD, AluInp.PREV_DELAY_0, AluInp.PREV_DELAY_4)
        dp[4].enable_delay_from_src(DelayInp.PREV_ALU_OUT, 1)  # b = vv/12
        dp[5].enable_alu(AluOp.ADD, AluInp.PREV_ALU_OUT, AluInp.PREV_DELAY_1)
        dp[6].pass_through_alu()
        dp[7].enable_alu(AluOp.MULTIPLY, AluInp.CURR_ALU_OUT, AluInp.PREV_ALU_OUT)
        u.enable_output(OutSel.ALU_OUT, OutPath.WR0_LO)
        return u

    def _ref(in0, in1, s0, s1, imm2):
        p = in0.shape[0]
        n = in1.reshape(p, -1, 2).shape[1]
        v = in0.reshape(p, n, 2)[:, :, 0].astype(np.float64)
        kk = in1.reshape(p, n, 2)[:, :, 0].astype(np.float64)
        c1 = 1.0 + v * s0 + v * v * s1
        gam = -1.0 + v * s0 - v * v * s1
        out = np.empty((p, n), np.float64)
        acc = np.ones(p)
        kprev = np.ones(p)
        for j in range(n):
            acc = c1[:, j] * (acc + kk[:, j] + gam[:, j] * kprev)
            kprev = kk[:, j]
            out[:, j] = acc
        return out.astype(np.float32)

    class _HandDveOp2(DveOp):
        def compile(self, ver):
            spec = DveOpSpec(
                name=self.name,
                opcode=dve_ops.get_dve_sub_opcode(self.name),
                uops=[_mk_init(), _mk_odd(), _mk_even()],
                rd1_en=True,
            )
            spec.validate(ver)
            return spec

    op = _HandDveOp2(
        name="VF_PAIR_SCAN_ANT",
        spec=Spec(body=Src0 * Src1 * C0 * C1, reference=_ref),
        subdim=False,
        uops_sha={},
    )
    dve_ops._SUB_OPCODE_FOR_NAME[op.name] = 1 + len(OPS)
    OPS.append(op)
    dve_ops.CUSTOM_DVE_SPECS[op.name] = op.spec
    return get_vf_scan_op()

# (len, start) chunks of coarse rows J: producers run chunk-by-chunk so the
# PDE row loop can start after the first chunk.
JCH = [(1, 0), (1, 1), (2, 2), (3, 4), (4, 7), (6, 11), (8, 17), (8, 25),
       (8, 33), (8, 41), (8, 49), (6, 57)]


def _build_program():
    from contextlib import ExitStack

    import concourse.tile as tile
    from concourse import bacc, mybir

    DBL = get_dbl_op()
    f32 = mybir.dt.float32
    bf16 = mybir.dt.bfloat16
    Act = mybir.ActivationFunctionType

    nc = bacc.Bacc(
        "TRN2",
        target_bir_lowering=False,
        debug=False,
        enable_asserts=True,
        num_devices=8,
    )
    dyh_d = nc.dram_tensor("dyh", [16, 63 * 32], bf16, kind="ExternalInput").ap()
    dxh_d = nc.dram_tensor("dxh", [16, 4 * 63], bf16, kind="ExternalInput").ap()
    dxl_d = nc.dram_tensor("dxl", [16, 4 * 63], bf16, kind="ExternalInput").ap()
    out_d = nc.dram_tensor("out", [4, 32], f32, kind="ExternalOutput").ap()

    with ExitStack() as ctx:
        tc = ctx.enter_context(tile.TileContext(nc))
        ws = ctx.enter_context(tc.tile_pool(name="ws", bufs=1))
        pp = ctx.enter_context(tc.tile_pool(name="pp", bufs=1, space="PSUM"))

        # Per-quadrant copies: group g's dy replica and dx slice live at
        # SBUF partitions 32g..32g+16 so the four matmul streams run as
        # independent diagonal PE tiles (tile_position (32g, 32g)).
        dxh_v = dxh_d.rearrange("d (g i) -> d g i", g=4)
        dxl_v = dxl_d.rearrange("d (g i) -> d g i", g=4)
        dyh_v = dyh_d.rearrange("d (j b) -> d j b", j=63)
        dxh_sb = ws.tile([128, 63], bf16)
        dxl_sb = ws.tile([128, 63], bf16)
        dyh_sb = ws.tile([128, 63, 32], bf16)
        # Distribute dma_start issue across engine queues: a DIRECT2D
        # issue costs ~600 ns on its sequencer, so 24 serial issues on Sync
        # would add ~14 us of head latency. Two waves: j 0:8 first so the
        # first matmul chunk's data lands fast.
        engs = [nc.sync, nc.scalar, nc.gpsimd]
        _ei = [0]

        def dma(out, in_):
            engs[_ei[0] % len(engs)].dma_start(out=out, in_=in_)
            _ei[0] += 1

        JW = 8
        for g in range(4):
            qs = slice(32 * g, 32 * g + 16)
            dma(dyh_sb[qs, 0:JW, :], dyh_v[:, 0:JW, :])
            dma(dxh_sb[qs, :], dxh_v[:, g, :])
            dma(dxl_sb[qs, :], dxl_v[:, g, :])
        for g in range(4):
            qs = slice(32 * g, 32 * g + 16)
            dma(dyh_sb[qs, JW:63, :], dyh_v[:, JW:63, :])

        # K-row ping-pong buffers; 1.0 everywhere gives both the j=0 boundary
        # column and the i=0 boundary row (row 0 of K is all ones).
        kbuf = ws.tile([128, 2, 128], f32)
        nc.vector.memset(kbuf[:, :, :], 1.0)

        ps = pp.tile([128, 63, 64], f32)  # coarse inc; row J at [:, J, 0:63]
        # ccc: coarse interleaved [gamma_J, c1_J] pairs per row (stride-2
        # writes from the poly ops). cc3: fine stream [gamma_j, c1_j, c1_j]
        # per fine column (Act-expanded; the double-row op's SRC_0).
        ccc = ws.tile([128, 63, 63], f32)
        cc3 = ws.tile([128, 63, 63, 2, 3], f32)
        C1P, GMP = get_poly_ops()

        for ln, st in JCH:
            jsl = slice(st, st + ln)
            for j in range(st, st + ln):
                for g in range(4):
                    po = slice(32 * g, 32 * g + 32)
                    qs = slice(32 * g, 32 * g + 16)
                    rh = dxh_sb[qs, :]
                    rl = dxl_sb[qs, :]
                    tp = (32 * g, 32 * g)
                    nc.tensor.matmul(
                        ps[po, j, 0:63], dyh_sb[qs, j, :], rh,
                        start=True, stop=False, tile_position=tp,
                    )
                    nc.tensor.matmul(
                        ps[po, j, 0:63], dyh_sb[qs, j, :], rl,
                        start=False, stop=True, tile_position=tp,
                    )
            nc.vector._custom_dve(
                GMP, out=ccc[:, jsl, :], in0=ps[:, jsl, 0:63],
                s0=0.5, s1=1.0 / 12.0, imm2=0.0,
            )

        # Act expansion to the fine [gamma, v, *] stream (the row op derives
        # c1 = v - gamma inline): gamma to (J, d, 0), v to (J, d, 1). Slot 2
        # is never read by any ALU (the C element's SRC_0 lane is unused), so
        # it stays unwritten. v reads PSUM directly (3-free-dim APs only).
        for ln, st in JCH:
            jsl = slice(st, st + ln)
            gsrc = ccc[:, jsl, :].unsqueeze(3).broadcast_to((128, ln, 63, 2))
            vsrc = ps[:, jsl, 0:63].unsqueeze(3).broadcast_to((128, ln, 63, 2))
            nc.scalar.activation(
                out=cc3[:, jsl, :, :, 0], in_=gsrc,
                func=Act.Copy, bias=0.0, scale=1.0,
            )
            nc.scalar.activation(
                out=cc3[:, jsl, :, :, 1], in_=vsrc,
                func=Act.Copy, bias=0.0, scale=1.0,
            )

        for ri in range(63):
            src = kbuf[:, ri & 1, :]
            dst = kbuf[:, 1 - (ri & 1), :]
            win = src[:, 1:127].unsqueeze(2).broadcast_to((128, 126, 3))
            cin = cc3[:, ri, :, :, :].rearrange("p j d s -> p (j d s)")
            nc.vector._custom_dve(
                DBL, out=dst[:, 1:127], in0=cin, in1=win,
                s0=0.0, s1=0.0, imm2=0.0,
            )

        # Compact the result column K[*, 126] (in kbuf[:, 0, :] after row 125)
        # into 4 partitions via a 32x32 block transpose, then one 4-descriptor
        # DMA: tt[32B + i, f] = kbuf[32B + f, 0, 96 + i] -> row i=30 holds col
        # 126 for the 32 pairs of partition-block B.
        # Compact result column K[*, 126] via 32x32 block transpose, then
        # four 1-descriptor DMAs (vs 128 4-byte descriptors, ~7 us).
        tt = ws.tile([128, 32], f32)
        nc.vector.transpose(tt[:], kbuf[:, 1, 96:128])
        oeng = [nc.scalar, nc.gpsimd, nc.sync, nc.scalar]
        for g in range(4):
            oeng[g].dma_start(
                out=out_d[g : g + 1, :], in_=tt[32 * g + 30 : 32 * g + 31, :]
            )

    nc.compile()
    return nc


def _get_nc():
    if "nc" not in _STATE:
        _STATE["nc"] = _build_program()
    return _STATE["nc"]


def _make_inputs(xs: np.ndarray, ys: np.ndarray):
    xs = np.asarray(xs, dtype=np.float32)
    ys = np.asarray(ys, dtype=np.float32)
    dxs_all = (xs[:, 1:, :] - xs[:, :-1, :]) * np.float32(0.25)  # (32, 63, 16)
    dys = ys[:, 1:, :] - ys[:, :-1, :]                           # (32, 63, 16)

    import ml_dtypes

    bf16 = ml_dtypes.bfloat16
    dysT = np.ascontiguousarray(dys.transpose(2, 1, 0))          # [d, j, b]
    dyh = np.ascontiguousarray(dysT.astype(bf16).reshape(16, 63 * 32))

    in_maps = []
    for c in range(8):
        dxs_c = np.ascontiguousarray(
            dxs_all[4 * c : 4 * c + 4].transpose(2, 0, 1).reshape(16, 4 * 63)
        )  # [d, (a', i)]
        dxh = dxs_c.astype(bf16)
        dxl = (dxs_c - dxh.astype(np.float32)).astype(bf16)
        in_maps.append({"dyh": dyh, "dxh": dxh, "dxl": dxl})
    return in_maps


def _run(nc, in_maps, **kwargs):
    from concourse.bass_utils import run_bass_kernel_spmd

    return run_bass_kernel_spmd(nc, in_maps, list(range(8)), **kwargs)


def kernel(xs: np.ndarray, ys: np.ndarray) -> np.ndarray:
    nc = _get_nc()
    in_maps = _make_inputs(xs, ys)
    res = _run(nc, in_maps)
    out = np.concatenate(
        [np.asarray(res.results[c]["out"]).reshape(4, 32) for c in range(8)], axis=0
    )
    return out.astype(np.float32)


# revision 27
# speedup vs baseline: 2292.2540x; 2292.2540x over previous
"""Trainium2 Bass kernel for the signature-kernel (Goursat PDE) problem.

Full inputs: xs (32, 64, 16) f32, ys (32, 64, 16) f32.
Output: (32, 32) f32 signature-kernel Gram matrix.

Strategy (8 NeuronCores, SPMD, no collectives):
  - Shard batch_x across cores: core c owns a in {4c..4c+3} -> 4*32 = 128
    (x, y) pairs, one pair per SBUF partition.
  - Coarse increments inc[a,b,I,J] = sum_d Dxs[a,I,d] Dys[b,J,d] / 4 are
    computed on-device with PE matmuls from DENSE hi/lo bf16 splits: per
    coarse row J and per a-group g, out partitions 32g..32g+32 contract
    lhsT = DysT[d, J, b] (16x32) against rhs = Dxs_g[d, I] (16x63); three
    accumulating matmuls (hi*hi + hi*lo + lo*hi) give fp32-level accuracy.
  - The Goursat PDE recurrence K[i+1,j+1] = c1*(K[i+1,j] + K[i,j+1])
    - c2*K[i,j] with the dyadically-refined (2x-duplicated) coefficients
    c1 = 1 + v/2 + v^2/12, c2 = 1 - v^2/12 is solved with ONE custom DVE
    instruction per ROW PAIR (63 instructions, get_dbl_op): dyadic
    refinement duplicates rows, so rows 2I and 2I+1 share coefficients
    and a hand-written 4-uop program advances BOTH in one pass at 1
    stream element/cycle -- u' = c1*(u + K0[j+1] + gamma*K0[j]),
    v' = c1*(v + u' + gamma*u) -- emitting only v (the intermediate row
    never materializes). gamma = -c2/c1 = -1 + v/2 - v^2/12 + O(v^4).
    The v accumulator crosses elements through slice 7's operand flop
    (alu_out_a_enable on the O element, NEXT_ALU_OUT_A read on C).
    Coefficients stream as [gamma_j, c1_j, c1_j] per fine column (379
    elements, ~553 ns per double row), produced per chunk by two Part-I
    custom poly ops (PSUM vf -> interleaved coarse pairs) and expanded
    by the otherwise-idle Act engine.
  - The final column K[*, 126] is compacted with a 32x32 StreamTranspose
    so the output DMA is 4 descriptors instead of 128.
"""

import os
import sys

import numpy as np

for _p in ("/opt/trn_rl_repo", "/root/.axon_site", "/root/.axon_site/_ro/trn_rl_repo",
           "/root/.axon_site/_ro/pypackages"):
    if os.path.isdir(_p) and _p not in sys.path:
        sys.path.append(_p)

_STATE: dict = {}
_OP2 = None

_OPS_CACHE: dict = {}


def _register_hand_op(name, make_uops, rd1, ref, spec_body):
    """Register a hand-written custom DVE op; returns the DveOp."""
    import concourse.dve_ops as dve_ops
    from concourse.dve_ops import DveOp, OPS
    from concourse.dve_spec import Spec
    from concourse.dve_uop import DveOpSpec

    for op in OPS:
        if op.name == name:
            return op

    class _H(DveOp):
        def compile(self, ver):
            spec = DveOpSpec(
                name=self.name,
                opcode=dve_ops.get_dve_sub_opcode(self.name),
                uops=make_uops(),
                rd1_en=rd1,
            )
            spec.validate(ver)
            return spec

    op = _H(name=name, spec=Spec(body=spec_body, reference=ref),
            subdim=False, uops_sha={})
    dve_ops._SUB_OPCODE_FOR_NAME[op.name] = 1 + len(OPS)
    OPS.append(op)
    dve_ops.CUSTOM_DVE_SPECS[op.name] = op.spec
    return op


def _register_spec_op(name, body, ref):
    """Register a Part-I Spec op (auto-lowered), bypassing the sha pin."""
    import concourse.dve_ops as dve_ops
    from concourse.dve_ops import DveOp, OPS
    from concourse.dve_spec import Spec, lower, _has_src1
    from concourse.dve_uop import DveOpSpec

    for op in OPS:
        if op.name == name:
            return op

    class _S(DveOp):
        def compile(self, ver):
            spec = DveOpSpec(
                name=self.name,
                opcode=dve_ops.get_dve_sub_opcode(self.name),
                uops=lower(self.spec, ver=ver),
                rd1_en=_has_src1(self.spec),
            )
            spec.validate(ver)
            return spec

    op = _S(name=name, spec=Spec(body=body, reference=ref),
            subdim=False, uops_sha={})
    dve_ops._SUB_OPCODE_FOR_NAME[op.name] = 1 + len(OPS)
    OPS.append(op)
    dve_ops.CUSTOM_DVE_SPECS[op.name] = op.spec
    return op


def get_poly_ops():
    """c1 = 1 + v*s0 + v^2*s1 ; gamma = -1 + v*s0 - v^2*s1 (= -c2/c1 +O(v^4))."""
    from concourse.dve_spec import Src0, C0, C1, One, sq

    c1 = _register_spec_op(
        "C1_POLY_ANT",
        Src0 * C0 + sq(Src0) * C1 + One,
        lambda in0, in1, s0, s1, imm2: (
            1.0 + in0.astype("float64") * s0 + in0.astype("float64") ** 2 * s1
        ).astype("float32"),
    )
    gm = _register_spec_op(
        "GAMMA_POLY_ANT",
        Src0 * C0 - sq(Src0) * C1 - One,
        lambda in0, in1, s0, s1, imm2: (
            in0.astype("float64") * s0 - in0.astype("float64") ** 2 * s1 - 1.0
        ).astype("float32"),
    )
    return c1, gm


def get_dbl_op():
    """Two PDE rows per instruction; see exp_custom3.py for the derivation.

    Stream (3 elements per fine column j): SRC_0 = [gamma_j, c1_j, c1_j],
    SRC_1 = K0[j+1] (x3). Per pair: u' = c1*(u + K0[j+1] + gamma*K0[j]),
    v' = c1*(v + u' + gamma*u); only v' is emitted (the intermediate row u
    never materializes). v is handed across elements via slice 7's a-flop
    (alu_out_a_enable on O, NEXT_ALU_OUT_A read on C at slice 6).
    """
    from concourse.dve_uop import (
        UopConfig, AluOp, AluInp, InpSel, OutSel, OutPath, Trigger, DelayInp,
    )
    import numpy as np

    def mk_init():
        u = UopConfig()
        u.enable_input(InpSel.ONE_F32, 1)
        u.repeat_count = 1
        u.trigger = (Trigger.COUNT, Trigger.NONE, Trigger.NONE)
        u.next_uop = (1, 0, 0)
        dp = u.datapath_config
        dp[0].enable_alu(AluOp.BYPASS, AluInp.PREV_DELAY_0)
        for b in range(7):
            dp[b].pass_through_delay(0)
        dp[5].enable_alu(AluOp.BYPASS, AluInp.PREV_DELAY_0)
        dp[7].enable_alu(AluOp.BYPASS, AluInp.PREV_DELAY_0)
        dp[7].alu_out_a_enable = 1
        return u

    def mk_O():
        u = UopConfig()
        u.enable_input(InpSel.SRC_0, 1)
        u.enable_input(InpSel.SRC_1, 2)
        u.require_inp0 = u.require_inp1 = 1
        u.repeat_count = 1
        u.trigger = (Trigger.SRC_TENSOR_DONE, Trigger.COUNT, Trigger.NONE)
        u.next_uop = (0, 2, 0)
        dp = u.datapath_config
        dp[0].enable_alu(AluOp.MULTIPLY, AluInp.PREV_DELAY_0, AluInp.CURR_ALU_OUT)
        dp[0].pass_through_delay(0, 1)
        dp[1].enable_alu(AluOp.ADD, AluInp.PREV_ALU_OUT, AluInp.PREV_DELAY_1)
        dp[1].pass_through_delay(0)
        dp[2].enable_delay_from_src(DelayInp.PREV_ALU_OUT, 1)
        dp[2].pass_through_delay(0)
        dp[3].enable_alu(AluOp.BYPASS, AluInp.PREV_DELAY_0)  # flop3 := gamma_j
        dp[3].pass_through_delay(0, 1)
        dp[4].pass_through_delay(0, 1)
        dp[5].enable_alu(AluOp.ADD, AluInp.PREV_DELAY_1, AluInp.CURR_ALU_OUT)
        dp[5].enable_delay_from_src(DelayInp.CURR_ALU_OUT, 2)
        dp[5].pass_through_delay(0)
        dp[6].enable_alu(AluOp.MULTIPLY, AluInp.PREV_DELAY_0, AluInp.PREV_DELAY_2)
        dp[7].enable_alu(AluOp.BYPASS, AluInp.CURR_ALU_OUT, AluInp.CURR_ALU_OUT)
        dp[7].alu_out_a_enable = 1
        return u

    def mk_E():
        u = UopConfig()
        u.enable_input(InpSel.SRC_0, 1)
        u.enable_input(InpSel.SRC_1, 2)
        u.require_inp0 = u.require_inp1 = 1
        u.repeat_count = 1
        u.trigger = (Trigger.SRC_TENSOR_DONE, Trigger.COUNT, Trigger.NONE)
        u.next_uop = (0, 3, 0)
        dp = u.datapath_config
        dp[0].enable_alu(AluOp.BYPASS, AluInp.PREV_DELAY_1)
        for b in range(3):
            dp[b].pass_through_delay(0)
        # s3: c1 = v - gamma (gamma parked in flop3 by O)
        dp[3].enable_alu(AluOp.SUBTRACT, AluInp.PREV_DELAY_0, AluInp.CURR_ALU_OUT)
        dp[4].pass_through_alu()  # carry c1
        dp[5].enable_alu(AluOp.MULTIPLY, AluInp.CURR_ALU_OUT, AluInp.PREV_ALU_OUT)
        dp[6].enable_alu(AluOp.ADD, AluInp.CURR_ALU_OUT, AluInp.PREV_ALU_OUT)
        return u

    def mk_C():
        u = UopConfig()
        u.enable_input(InpSel.SRC_0, 1)
        u.enable_input(InpSel.SRC_1, 2)
        u.require_inp0 = u.require_inp1 = 1
        u.repeat_count = 1
        u.trigger = (Trigger.SRC_TENSOR_DONE, Trigger.COUNT, Trigger.NONE)
        u.next_uop = (0, 1, 0)
        dp = u.datapath_config
        for b in range(6):
            dp[b].pass_through_delay(0)
        # s3: pick up c1 (flop3, written by E one cycle earlier) into lane1
        dp[3].enable_delay_from_src(DelayInp.CURR_ALU_OUT, 1)
        dp[4].pass_through_delay(1)
        dp[5].pass_through_delay(1)
        dp[6].enable_alu(AluOp.ADD, AluInp.CURR_ALU_OUT, AluInp.NEXT_ALU_OUT_A)
        dp[6].pass_through_delay(0, 1)
        dp[7].enable_alu(AluOp.MULTIPLY, AluInp.PREV_ALU_OUT, AluInp.PREV_DELAY_1)
        u.enable_output(OutSel.ALU_OUT, OutPath.WR0_LO)
        return u

    def ref(in0, in1, s0, s1, imm2):
        p = in0.shape[0]
        n = in0.shape[-1] // 3
        cc = in0.reshape(p, n, 3).astype(np.float64)
        kk = in1.reshape(p, n, 3)[:, :, 0].astype(np.float64)
        gam = cc[:, :, 0]
        c1 = cc[:, :, 1] - gam  # slot 1 carries v; c1 = v - gamma
        out = np.empty((p, n), np.float64)
        u = np.ones(p); v = np.ones(p); kprev = np.ones(p)
        for j in range(n):
            un = c1[:, j] * (u + kk[:, j] + gam[:, j] * kprev)
            v = c1[:, j] * (v + un + gam[:, j] * u)
            u, kprev = un, kk[:, j]
            out[:, j] = v
        return out.astype(np.float32)

    from concourse.dve_spec import Src0, Src1

    return _register_hand_op(
        "DBL_PAIR_SCAN_ANT", lambda: [mk_init(), mk_O(), mk_E(), mk_C()],
        True, ref, Src0 * Src1,
    )




def get_vf_scan_op():
    """Custom DVE op VF_PAIR_SCAN_ANT (hand-written 3-uop program).

    Reads the coarse vf value v_j (duplicated 4x via a stride-0 AP) and the
    previous K row (K[j+1] duplicated 2x), computes c1/gamma inline:
        c1 = 1 + v/2 + v^2/12,  gamma = -1 + v/2 - v^2/12  (= -c2/c1 + O(v^4))
        acc_j = c1_j * (acc_{j-1} + K[j+1] + gamma_j * K[j])
    and emits acc_j (= K_new[j+1]) on every second stream element, at one
    stream element per cycle. s0 = 0.5 (CONST_0), s1 = 1/12 (CONST_1).
    """
    global _OP2
    if _OP2 is not None:
        return _OP2
    import concourse.dve_ops as dve_ops
    from concourse.dve_ops import DveOp, OPS
    from concourse.dve_spec import Spec, Src0, Src1, C0, C1
    from concourse.dve_uop import (
        DveOpSpec,
        UopConfig,
        AluOp,
        AluInp,
        InpSel,
        OutSel,
        OutPath,
        Trigger,
        DelayInp,
    )

    for op in OPS:
        if op.name == "VF_PAIR_SCAN_ANT":
            _OP2 = op
            return op

    def _inputs(u):
        u.enable_input(InpSel.SRC_0, 0)    # v -> blk0 PREV_ALU_OUT
        u.enable_input(InpSel.SRC_0, 1)    # v -> lane 0
        u.enable_input(InpSel.SRC_1, 2)    # K[j+1] -> lane 1
        u.enable_input(InpSel.CONST_0, 3)  # 0.5 -> lane 2
        u.enable_input(InpSel.CONST_1, 4)  # 1/12 -> lane 3
        u.enable_input(InpSel.ONE_F32, 5)  # 1.0 -> lane 4
        u.require_inp0 = u.require_inp1 = 1
        u.repeat_count = 1
        return u

    def _mk_init():
        u = UopConfig()
        u.enable_input(InpSel.ONE_F32, 1)  # lane 0
        u.repeat_count = 1
        u.trigger = (Trigger.COUNT, Trigger.NONE, Trigger.NONE)
        u.next_uop = (1, 0, 0)
        dp = u.datapath_config
        dp[0].pass_through_delay(0)
        dp[1].enable_alu(AluOp.BYPASS, AluInp.PREV_DELAY_0)  # flop1 := 1.0 (K[0])
        for b in (1, 2, 3, 4, 5, 6):
            dp[b].pass_through_delay(0)
        dp[7].enable_alu(AluOp.BYPASS, AluInp.PREV_DELAY_0)  # flop7 := 1.0 (acc)
        return u

    def _mk_odd():
        # even stream elements (2j): compute gamma_j, p, m, u
        u = _inputs(UopConfig())
        u.trigger = (Trigger.SRC_TENSOR_DONE, Trigger.COUNT, Trigger.NONE)
        u.next_uop = (0, 2, 0)
        dp = u.datapath_config
        dp[0].enable_alu(AluOp.MULTIPLY, AluInp.PREV_ALU_OUT, AluInp.PREV_DELAY_2)
        dp[0].pass_through_delay(0, 1, 3, 4)
        dp[1].enable_alu(AluOp.MULTIPLY, AluInp.PREV_DELAY_0, AluInp.PREV_DELAY_0)
        dp[1].enable_delay_from_src(DelayInp.CURR_ALU_OUT, 5)  # K[j] handoff read
        dp[1].enable_delay_from_src(DelayInp.PREV_ALU_OUT, 0)  # a = v/2
        dp[1].pass_through_delay(1, 3, 4)
        dp[2].enable_alu(AluOp.MULTIPLY, AluInp.PREV_ALU_OUT, AluInp.PREV_DELAY_3)
        dp[2].pass_through_delay(0, 1, 4, 5)
        dp[3].enable_alu(AluOp.SUBTRACT, AluInp.PREV_DELAY_0, AluInp.PREV_DELAY_4)
        dp[3].enable_delay_from_src(DelayInp.PREV_ALU_OUT, 0)  # b = vv/12
        dp[3].pass_through_delay(1, 5)
        dp[4].enable_alu(AluOp.SUBTRACT, AluInp.PREV_ALU_OUT, AluInp.PREV_DELAY_0)
        dp[4].pass_through_delay(1, 5)
        dp[5].enable_alu(AluOp.MULTIPLY, AluInp.PREV_ALU_OUT, AluInp.PREV_DELAY_5)
        dp[5].pass_through_delay(1)
        dp[6].enable_alu(AluOp.ADD, AluInp.PREV_ALU_OUT, AluInp.PREV_DELAY_1)
        dp[7].enable_alu(AluOp.ADD, AluInp.PREV_ALU_OUT, AluInp.CURR_ALU_OUT)
        return u

    def _mk_even():
        # odd stream elements (2j+1): c1_j, K handoff, acc = u * c1; emit
        u = _inputs(UopConfig())
        u.trigger = (Trigger.SRC_TENSOR_DONE, Trigger.COUNT, Trigger.NONE)
        u.next_uop = (0, 1, 0)
        dp = u.datapath_config
        dp[0].enable_alu(AluOp.MULTIPLY, AluInp.PREV_ALU_OUT, AluInp.PREV_DELAY_0)
        dp[0].pass_through_delay(0, 1, 2, 3, 4)
        dp[1].enable_alu(AluOp.BYPASS, AluInp.PREV_DELAY_1)   # flop1 := K[j+1]
        dp[1].enable_delay_from_src(DelayInp.PREV_ALU_OUT, 5)  # vv
        dp[1].pass_through_delay(0, 2, 3, 4)
        dp[2].enable_alu(AluOp.MULTIPLY, AluInp.PREV_DELAY_0, AluInp.PREV_DELAY_2)
        dp[2].pass_through_delay(3, 4, 5)
        dp[3].enable_alu(AluOp.MULTIPLY, AluInp.PREV_DELAY_5, AluInp.PREV_DELAY_3)
        dp[3].enable_delay_from_src(DelayInp.PREV_ALU_OUT, 0)  # a = v/2
        dp[3].pass_through_delay(4)
        dp[4].enable_alu(AluOp.ADD, AluInp.PREV_DELAY_0, AluInp.PREV_DELAY_4)
        dp[4].enable_delay_from_src(DelayInp.PREV_ALU_OUT, 1)  # b = vv/12
        dp[5].enable_alu(AluOp.ADD, AluInp.PREV_ALU_OUT, AluInp.PREV_DELAY_1)
        dp[6].pass_through_alu()
        dp[7].enable_alu(AluOp.MULTIPLY, AluInp.CURR_ALU_OUT, AluInp.PREV_ALU_OUT)
        u.enable_output(OutSel.ALU_OUT, OutPath.WR0_LO)
        return u

    def _ref(in0, in1, s0, s1, imm2):
        p = in0.shape[0]
        n = in1.reshape(p, -1, 2).shape[1]
        v = in0.reshape(p, n, 2)[:, :, 0].astype(np.float64)
        kk = in1.reshape(p, n, 2)[:, :, 0].astype(np.float64)
        c1 = 1.0 + v * s0 + v * v * s1
        gam = -1.0 + v * s0 - v * v * s1
        out = np.empty((p, n), np.float64)
        acc = np.ones(p)
        kprev = np.ones(p)
        for j in range(n):
            acc = c1[:, j] * (acc + kk[:, j] + gam[:, j] * kprev)
            kprev = kk[:, j]
            out[:, j] = acc
        return out.astype(np.float32)

    class _HandDveOp2(DveOp):
        def compile(self, ver):
            spec = DveOpSpec(
                name=self.name,
                opcode=dve_ops.get_dve_sub_opcode(self.name),
                uops=[_mk_init(), _mk_odd(), _mk_even()],
                rd1_en=True,
            )
            spec.validate(ver)
            return spec

    op = _HandDveOp2(
        name="VF_PAIR_SCAN_ANT",
        spec=Spec(body=Src0 * Src1 * C0 * C1, reference=_ref),
        subdim=False,
        uops_sha={},
    )
    dve_ops._SUB_OPCODE_FOR_NAME[op.name] = 1 + len(OPS)
    OPS.append(op)
    dve_ops.CUSTOM_DVE_SPECS[op.name] = op.spec
    return get_vf_scan_op()

# (len, start) chunks of coarse rows J: producers run chunk-by-chunk so the
# PDE row loop can start after the first chunk.
JCH = [(1, 0), (1, 1), (2, 2), (3, 4), (4, 7), (6, 11), (8, 17), (8, 25),
       (8, 33), (8, 41), (8, 49), (6, 57)]


def _build_program():
    from contextlib import ExitStack

    import concourse.tile as tile
    from concourse import bacc, mybir

    DBL = get_dbl_op()
    f32 = mybir.dt.float32
    bf16 = mybir.dt.bfloat16
    Act = mybir.ActivationFunctionType

    nc = bacc.Bacc(
        "TRN2",
        target_bir_lowering=False,
        debug=False,
        enable_asserts=True,
        num_devices=8,
    )
    dyh_d = nc.dram_tensor("dyh", [32, 63 * 32], bf16, kind="ExternalInput").ap()
    dxh_d = nc.dram_tensor("dxh", [32, 4 * 63], bf16, kind="ExternalInput").ap()
    out_d = nc.dram_tensor("out", [4, 32], f32, kind="ExternalOutput").ap()

    with ExitStack() as ctx:
        tc = ctx.enter_context(tile.TileContext(nc))
        ws = ctx.enter_context(tc.tile_pool(name="ws", bufs=1))
        pp = ctx.enter_context(tc.tile_pool(name="pp", bufs=1, space="PSUM"))

        # Per-quadrant copies: group g's dy replica and dx slice live at
        # SBUF partitions 32g..32g+16 so the four matmul streams run as
        # independent diagonal PE tiles (tile_position (32g, 32g)).
        dxh_v = dxh_d.rearrange("d (g i) -> d g i", g=4)
        dyh_v = dyh_d.rearrange("d (j b) -> d j b", j=63)
        dxh_sb = ws.tile([128, 63], bf16)
        dyh_sb = ws.tile([128, 63, 32], bf16)
        # Distribute dma_start issue across engine queues: a DIRECT2D
        # issue costs ~600 ns on its sequencer, so 24 serial issues on Sync
        # would add ~14 us of head latency. Two waves: j 0:8 first so the
        # first matmul chunk's data lands fast.
        engs = [nc.sync, nc.scalar, nc.gpsimd]
        _ei = [0]

        def dma(out, in_):
            engs[_ei[0] % len(engs)].dma_start(out=out, in_=in_)
            _ei[0] += 1

        JW = 8
        for g in range(4):
            qs = slice(32 * g, 32 * g + 32)
            dma(dyh_sb[qs, 0:JW, :], dyh_v[:, 0:JW, :])
            dma(dxh_sb[qs, :], dxh_v[:, g, :])
        for g in range(4):
            qs = slice(32 * g, 32 * g + 32)
            dma(dyh_sb[qs, JW:63, :], dyh_v[:, JW:63, :])

        # K-row ping-pong buffers; 1.0 everywhere gives both the j=0 boundary
        # column and the i=0 boundary row (row 0 of K is all ones).
        kbuf = ws.tile([128, 2, 128], f32)
        nc.vector.memset(kbuf[:, :, :], 1.0)

        ps = pp.tile([128, 63, 64], f32)  # coarse inc; row J at [:, J, 0:63]
        # ccc: coarse interleaved [gamma_J, c1_J] pairs per row (stride-2
        # writes from the poly ops). cc3: fine stream [gamma_j, c1_j, c1_j]
        # per fine column (Act-expanded; the double-row op's SRC_0).
        ccc = ws.tile([128, 63, 63], f32)
        cc3 = ws.tile([128, 63, 63, 2, 3], f32)
        C1P, GMP = get_poly_ops()

        for ln, st in JCH:
            jsl = slice(st, st + ln)
            for j in range(st, st + ln):
                for g in range(4):
                    po = slice(32 * g, 32 * g + 32)
                    qs = slice(32 * g, 32 * g + 32)
                    tp = (32 * g, 32 * g)
                    # K=32 contraction-concatenation: [dyh; dyh] x [dxh; dxl]
                    # = dyh*dxh + dyh*dxl in ONE matmul (half the ldweights).
                    nc.tensor.matmul(
                        ps[po, j, 0:63], dyh_sb[qs, j, :], dxh_sb[qs, :],
                        start=True, stop=True, tile_position=tp,
                    )
            nc.vector._custom_dve(
                GMP, out=ccc[:, jsl, :], in0=ps[:, jsl, 0:63],
                s0=0.5, s1=1.0 / 12.0, imm2=0.0,
            )

        # Act expansion to the fine [gamma, v, *] stream (the row op derives
        # c1 = v - gamma inline): gamma to (J, d, 0), v to (J, d, 1). Slot 2
        # is never read by any ALU (the C element's SRC_0 lane is unused), so
        # it stays unwritten. v reads PSUM directly (3-free-dim APs only).
        for ln, st in JCH:
            jsl = slice(st, st + ln)
            gsrc = ccc[:, jsl, :].unsqueeze(3).broadcast_to((128, ln, 63, 2))
            vsrc = ps[:, jsl, 0:63].unsqueeze(3).broadcast_to((128, ln, 63, 2))
            nc.scalar.activation(
                out=cc3[:, jsl, :, :, 0], in_=gsrc,
                func=Act.Copy, bias=0.0, scale=1.0,
            )
            nc.scalar.activation(
                out=cc3[:, jsl, :, :, 1], in_=vsrc,
                func=Act.Copy, bias=0.0, scale=1.0,
            )

        for ri in range(63):
            src = kbuf[:, ri & 1, :]
            dst = kbuf[:, 1 - (ri & 1), :]
            win = src[:, 1:127].unsqueeze(2).broadcast_to((128, 126, 3))
            cin = cc3[:, ri, :, :, :].rearrange("p j d s -> p (j d s)")
            nc.vector._custom_dve(
                DBL, out=dst[:, 1:127], in0=cin, in1=win,
                s0=0.0, s1=0.0, imm2=0.0,
            )

        # Compact the result column K[*, 126] (in kbuf[:, 0, :] after row 125)
        # into 4 partitions via a 32x32 block transpose, then one 4-descriptor
        # DMA: tt[32B + i, f] = kbuf[32B + f, 0, 96 + i] -> row i=30 holds col
        # 126 for the 32 pairs of partition-block B.
        # Compact result column K[*, 126] via 32x32 block transpose, then
        # four 1-descriptor DMAs (vs 128 4-byte descriptors, ~7 us).
        tt = ws.tile([128, 32], f32)
        nc.vector.transpose(tt[:], kbuf[:, 1, 96:128])
        oeng = [nc.scalar, nc.gpsimd, nc.sync, nc.scalar]
        for g in range(4):
            oeng[g].dma_start(
                out=out_d[g : g + 1, :], in_=tt[32 * g + 30 : 32 * g + 31, :]
            )

    nc.compile()
    return nc


def _get_nc():
    if "nc" not in _STATE:
        _STATE["nc"] = _build_program()
    return _STATE["nc"]


def _make_inputs(xs: np.ndarray, ys: np.ndarray):
    xs = np.asarray(xs, dtype=np.float32)
    ys = np.asarray(ys, dtype=np.float32)
    dxs_all = (xs[:, 1:, :] - xs[:, :-1, :]) * np.float32(0.25)  # (32, 63, 16)
    dys = ys[:, 1:, :] - ys[:, :-1, :]                           # (32, 63, 16)

    import ml_dtypes

    bf16 = ml_dtypes.bfloat16
    dysT = np.ascontiguousarray(dys.transpose(2, 1, 0))          # [d, j, b]
    dyh1 = dysT.astype(bf16).reshape(16, 63 * 32)
    dyh = np.ascontiguousarray(np.concatenate([dyh1, dyh1], axis=0))

    in_maps = []
    for c in range(8):
        dxs_c = np.ascontiguousarray(
            dxs_all[4 * c : 4 * c + 4].transpose(2, 0, 1).reshape(16, 4 * 63)
        )  # [d, (a', i)]
        dxhh = dxs_c.astype(bf16)
        dxll = (dxs_c - dxhh.astype(np.float32)).astype(bf16)
        dxh = np.ascontiguousarray(np.concatenate([dxhh, dxll], axis=0))
        in_maps.append({"dyh": dyh, "dxh": dxh})
    return in_maps


def _run(nc, in_maps, **kwargs):
    from concourse.bass_utils import run_bass_kernel_spmd

    return run_bass_kernel_spmd(nc, in_maps, list(range(8)), **kwargs)


def kernel(xs: np.ndarray, ys: np.ndarray) -> np.ndarray:
    nc = _get_nc()
    in_maps = _make_inputs(xs, ys)
    res = _run(nc, in_maps)
    out = np.concatenate(
        [np.asarray(res.results[c]["out"]).reshape(4, 32) for c in range(8)], axis=0
    )
    return out.astype(np.float32)


# revision 28
# speedup vs baseline: 2334.8950x; 1.0186x over previous
"""Trainium2 Bass kernel for the signature-kernel (Goursat PDE) problem.

Full inputs: xs (32, 64, 16) f32, ys (32, 64, 16) f32.
Output: (32, 32) f32 signature-kernel Gram matrix.

Strategy (8 NeuronCores, SPMD, no collectives):
  - Shard batch_x across cores: core c owns a in {4c..4c+3} -> 4*32 = 128
    (x, y) pairs, one pair per SBUF partition.
  - Coarse increments inc[a,b,I,J] = sum_d Dxs[a,I,d] Dys[b,J,d] / 4 are
    computed on-device with PE matmuls from DENSE hi/lo bf16 splits: per
    coarse row J and per a-group g, out partitions 32g..32g+32 contract
    lhsT = DysT[d, J, b] (16x32) against rhs = Dxs_g[d, I] (16x63); three
    accumulating matmuls (hi*hi + hi*lo + lo*hi) give fp32-level accuracy.
  - The Goursat PDE recurrence K[i+1,j+1] = c1*(K[i+1,j] + K[i,j+1])
    - c2*K[i,j] with the dyadically-refined (2x-duplicated) coefficients
    c1 = 1 + v/2 + v^2/12, c2 = 1 - v^2/12 is solved with ONE custom DVE
    instruction per ROW PAIR (63 instructions, get_dbl_op): dyadic
    refinement duplicates rows, so rows 2I and 2I+1 share coefficients
    and a hand-written 4-uop program advances BOTH in one pass at 1
    stream element/cycle -- u' = c1*(u + K0[j+1] + gamma*K0[j]),
    v' = c1*(v + u' + gamma*u) -- emitting only v (the intermediate row
    never materializes). gamma = -c2/c1 = -1 + v/2 - v^2/12 + O(v^4).
    The v accumulator crosses elements through slice 7's operand flop
    (alu_out_a_enable on the O element, NEXT_ALU_OUT_A read on C).
    Coefficients stream as [gamma_j, c1_j, c1_j] per fine column (379
    elements, ~553 ns per double row), produced per chunk by two Part-I
    custom poly ops (PSUM vf -> interleaved coarse pairs) and expanded
    by the otherwise-idle Act engine.
  - The final column K[*, 126] is compacted with a 32x32 StreamTranspose
    so the output DMA is 4 descriptors instead of 128.
"""

import os
import sys

import numpy as np

for _p in ("/opt/trn_rl_repo", "/root/.axon_site", "/root/.axon_site/_ro/trn_rl_repo",
           "/root/.axon_site/_ro/pypackages"):
    if os.path.isdir(_p) and _p not in sys.path:
        sys.path.append(_p)

_STATE: dict = {}
_OP2 = None

_OPS_CACHE: dict = {}


def _register_hand_op(name, make_uops, rd1, ref, spec_body):
    """Register a hand-written custom DVE op; returns the DveOp."""
    import concourse.dve_ops as dve_ops
    from concourse.dve_ops import DveOp, OPS
    from concourse.dve_spec import Spec
    from concourse.dve_uop import DveOpSpec

    for op in OPS:
        if op.name == name:
            return op

    class _H(DveOp):
        def compile(self, ver):
            spec = DveOpSpec(
                name=self.name,
                opcode=dve_ops.get_dve_sub_opcode(self.name),
                uops=make_uops(),
                rd1_en=rd1,
            )
            spec.validate(ver)
            return spec

    op = _H(name=name, spec=Spec(body=spec_body, reference=ref),
            subdim=False, uops_sha={})
    dve_ops._SUB_OPCODE_FOR_NAME[op.name] = 1 + len(OPS)
    OPS.append(op)
    dve_ops.CUSTOM_DVE_SPECS[op.name] = op.spec
    return op


def _register_spec_op(name, body, ref):
    """Register a Part-I Spec op (auto-lowered), bypassing the sha pin."""
    import concourse.dve_ops as dve_ops
    from concourse.dve_ops import DveOp, OPS
    from concourse.dve_spec import Spec, lower, _has_src1
    from concourse.dve_uop import DveOpSpec

    for op in OPS:
        if op.name == name:
            return op

    class _S(DveOp):
        def compile(self, ver):
            spec = DveOpSpec(
                name=self.name,
                opcode=dve_ops.get_dve_sub_opcode(self.name),
                uops=lower(self.spec, ver=ver),
                rd1_en=_has_src1(self.spec),
            )
            spec.validate(ver)
            return spec

    op = _S(name=name, spec=Spec(body=body, reference=ref),
            subdim=False, uops_sha={})
    dve_ops._SUB_OPCODE_FOR_NAME[op.name] = 1 + len(OPS)
    OPS.append(op)
    dve_ops.CUSTOM_DVE_SPECS[op.name] = op.spec
    return op


def get_poly_ops():
    """c1 = 1 + v*s0 + v^2*s1 ; gamma = -1 + v*s0 - v^2*s1 (= -c2/c1 +O(v^4))."""
    from concourse.dve_spec import Src0, C0, C1, One, sq

    c1 = _register_spec_op(
        "C1_POLY_ANT",
        Src0 * C0 + sq(Src0) * C1 + One,
        lambda in0, in1, s0, s1, imm2: (
            1.0 + in0.astype("float64") * s0 + in0.astype("float64") ** 2 * s1
        ).astype("float32"),
    )
    gm = _register_spec_op(
        "GAMMA_POLY_ANT",
        Src0 * C0 - sq(Src0) * C1 - One,
        lambda in0, in1, s0, s1, imm2: (
            in0.astype("float64") * s0 - in0.astype("float64") ** 2 * s1 - 1.0
        ).astype("float32"),
    )
    return c1, gm


def get_dbl_op():
    """Two PDE rows per instruction; see exp_custom3.py for the derivation.

    Stream (3 elements per fine column j): SRC_0 = [gamma_j, c1_j, c1_j],
    SRC_1 = K0[j+1] (x3). Per pair: u' = c1*(u + K0[j+1] + gamma*K0[j]),
    v' = c1*(v + u' + gamma*u); only v' is emitted (the intermediate row u
    never materializes). v is handed across elements via slice 7's a-flop
    (alu_out_a_enable on O, NEXT_ALU_OUT_A read on C at slice 6).
    """
    from concourse.dve_uop import (
        UopConfig, AluOp, AluInp, InpSel, OutSel, OutPath, Trigger, DelayInp,
    )
    import numpy as np

    def mk_init():
        u = UopConfig()
        u.enable_input(InpSel.ONE_F32, 1)
        u.repeat_count = 1
        u.trigger = (Trigger.COUNT, Trigger.NONE, Trigger.NONE)
        u.next_uop = (1, 0, 0)
        dp = u.datapath_config
        dp[0].enable_alu(AluOp.BYPASS, AluInp.PREV_DELAY_0)
        for b in range(7):
            dp[b].pass_through_delay(0)
        dp[5].enable_alu(AluOp.BYPASS, AluInp.PREV_DELAY_0)
        dp[7].enable_alu(AluOp.BYPASS, AluInp.PREV_DELAY_0)
        dp[7].alu_out_a_enable = 1
        return u

    def mk_O():
        u = UopConfig()
        u.enable_input(InpSel.SRC_0, 1)
        u.enable_input(InpSel.SRC_1, 2)
        u.require_inp0 = u.require_inp1 = 1
        u.repeat_count = 1
        u.trigger = (Trigger.SRC_TENSOR_DONE, Trigger.COUNT, Trigger.NONE)
        u.next_uop = (0, 2, 0)
        dp = u.datapath_config
        dp[0].enable_alu(AluOp.MULTIPLY, AluInp.PREV_DELAY_0, AluInp.CURR_ALU_OUT)
        dp[0].pass_through_delay(0, 1)
        dp[1].enable_alu(AluOp.ADD, AluInp.PREV_ALU_OUT, AluInp.PREV_DELAY_1)
        dp[1].pass_through_delay(0)
        dp[2].enable_delay_from_src(DelayInp.PREV_ALU_OUT, 1)
        dp[2].pass_through_delay(0)
        dp[3].enable_alu(AluOp.BYPASS, AluInp.PREV_DELAY_0)  # flop3 := gamma_j
        dp[3].pass_through_delay(0, 1)
        dp[4].pass_through_delay(0, 1)
        dp[5].enable_alu(AluOp.ADD, AluInp.PREV_DELAY_1, AluInp.CURR_ALU_OUT)
        dp[5].enable_delay_from_src(DelayInp.CURR_ALU_OUT, 2)
        dp[5].pass_through_delay(0)
        dp[6].enable_alu(AluOp.MULTIPLY, AluInp.PREV_DELAY_0, AluInp.PREV_DELAY_2)
        dp[7].enable_alu(AluOp.BYPASS, AluInp.CURR_ALU_OUT, AluInp.CURR_ALU_OUT)
        dp[7].alu_out_a_enable = 1
        return u

    def mk_E():
        u = UopConfig()
        u.enable_input(InpSel.SRC_0, 1)
        u.enable_input(InpSel.SRC_1, 2)
        u.require_inp0 = u.require_inp1 = 1
        u.repeat_count = 1
        u.trigger = (Trigger.SRC_TENSOR_DONE, Trigger.COUNT, Trigger.NONE)
        u.next_uop = (0, 3, 0)
        dp = u.datapath_config
        dp[0].enable_alu(AluOp.BYPASS, AluInp.PREV_DELAY_1)
        for b in range(3):
            dp[b].pass_through_delay(0)
        # s3: c1 = v - gamma (gamma parked in flop3 by O)
        dp[3].enable_alu(AluOp.SUBTRACT, AluInp.PREV_DELAY_0, AluInp.CURR_ALU_OUT)
        dp[4].pass_through_alu()  # carry c1
        dp[5].enable_alu(AluOp.MULTIPLY, AluInp.CURR_ALU_OUT, AluInp.PREV_ALU_OUT)
        dp[6].enable_alu(AluOp.ADD, AluInp.CURR_ALU_OUT, AluInp.PREV_ALU_OUT)
        return u

    def mk_C():
        u = UopConfig()
        u.enable_input(InpSel.SRC_0, 1)
        u.enable_input(InpSel.SRC_1, 2)
        u.require_inp0 = u.require_inp1 = 1
        u.repeat_count = 1
        u.trigger = (Trigger.SRC_TENSOR_DONE, Trigger.COUNT, Trigger.NONE)
        u.next_uop = (0, 1, 0)
        dp = u.datapath_config
        for b in range(6):
            dp[b].pass_through_delay(0)
        # s3: pick up c1 (flop3, written by E one cycle earlier) into lane1
        dp[3].enable_delay_from_src(DelayInp.CURR_ALU_OUT, 1)
        dp[4].pass_through_delay(1)
        dp[5].pass_through_delay(1)
        dp[6].enable_alu(AluOp.ADD, AluInp.CURR_ALU_OUT, AluInp.NEXT_ALU_OUT_A)
        dp[6].pass_through_delay(0, 1)
        dp[7].enable_alu(AluOp.MULTIPLY, AluInp.PREV_ALU_OUT, AluInp.PREV_DELAY_1)
        u.enable_output(OutSel.ALU_OUT, OutPath.WR0_LO)
        return u

    def ref(in0, in1, s0, s1, imm2):
        p = in0.shape[0]
        n = in0.shape[-1] // 3
        cc = in0.reshape(p, n, 3).astype(np.float64)
        kk = in1.reshape(p, n, 3)[:, :, 0].astype(np.float64)
        gam = cc[:, :, 0]
        c1 = cc[:, :, 1] - gam  # slot 1 carries v; c1 = v - gamma
        out = np.empty((p, n), np.float64)
        u = np.ones(p); v = np.ones(p); kprev = np.ones(p)
        for j in range(n):
            un = c1[:, j] * (u + kk[:, j] + gam[:, j] * kprev)
            v = c1[:, j] * (v + un + gam[:, j] * u)
            u, kprev = un, kk[:, j]
            out[:, j] = v
        return out.astype(np.float32)

    from concourse.dve_spec import Src0, Src1

    return _register_hand_op(
        "DBL_PAIR_SCAN_ANT", lambda: [mk_init(), mk_O(), mk_E(), mk_C()],
        True, ref, Src0 * Src1,
    )




def get_vf_scan_op():
    """Custom DVE op VF_PAIR_SCAN_ANT (hand-written 3-uop program).

    Reads the coarse vf value v_j (duplicated 4x via a stride-0 AP) and the
    previous K row (K[j+1] duplicated 2x), computes c1/gamma inline:
        c1 = 1 + v/2 + v^2/12,  gamma = -1 + v/2 - v^2/12  (= -c2/c1 + O(v^4))
        acc_j = c1_j * (acc_{j-1} + K[j+1] + gamma_j * K[j])
    and emits acc_j (= K_new[j+1]) on every second stream element, at one
    stream element per cycle. s0 = 0.5 (CONST_0), s1 = 1/12 (CONST_1).
    """
    global _OP2
    if _OP2 is not None:
        return _OP2
    import concourse.dve_ops as dve_ops
    from concourse.dve_ops import DveOp, OPS
    from concourse.dve_spec import Spec, Src0, Src1, C0, C1
    from concourse.dve_uop import (
        DveOpSpec,
        UopConfig,
        AluOp,
        AluInp,
        InpSel,
        OutSel,
        OutPath,
        Trigger,
        DelayInp,
    )

    for op in OPS:
        if op.name == "VF_PAIR_SCAN_ANT":
            _OP2 = op
            return op

    def _inputs(u):
        u.enable_input(InpSel.SRC_0, 0)    # v -> blk0 PREV_ALU_OUT
        u.enable_input(InpSel.SRC_0, 1)    # v -> lane 0
        u.enable_input(InpSel.SRC_1, 2)    # K[j+1] -> lane 1
        u.enable_input(InpSel.CONST_0, 3)  # 0.5 -> lane 2
        u.enable_input(InpSel.CONST_1, 4)  # 1/12 -> lane 3
        u.enable_input(InpSel.ONE_F32, 5)  # 1.0 -> lane 4
        u.require_inp0 = u.require_inp1 = 1
        u.repeat_count = 1
        return u

    def _mk_init():
        u = UopConfig()
        u.enable_input(InpSel.ONE_F32, 1)  # lane 0
        u.repeat_count = 1
        u.trigger = (Trigger.COUNT, Trigger.NONE, Trigger.NONE)
        u.next_uop = (1, 0, 0)
        dp = u.datapath_config
        dp[0].pass_through_delay(0)
        dp[1].enable_alu(AluOp.BYPASS, AluInp.PREV_DELAY_0)  # flop1 := 1.0 (K[0])
        for b in (1, 2, 3, 4, 5, 6):
            dp[b].pass_through_delay(0)
        dp[7].enable_alu(AluOp.BYPASS, AluInp.PREV_DELAY_0)  # flop7 := 1.0 (acc)
        return u

    def _mk_odd():
        # even stream elements (2j): compute gamma_j, p, m, u
        u = _inputs(UopConfig())
        u.trigger = (Trigger.SRC_TENSOR_DONE, Trigger.COUNT, Trigger.NONE)
        u.next_uop = (0, 2, 0)
        dp = u.datapath_config
        dp[0].enable_alu(AluOp.MULTIPLY, AluInp.PREV_ALU_OUT, AluInp.PREV_DELAY_2)
        dp[0].pass_through_delay(0, 1, 3, 4)
        dp[1].enable_alu(AluOp.MULTIPLY, AluInp.PREV_DELAY_0, AluInp.PREV_DELAY_0)
        dp[1].enable_delay_from_src(DelayInp.CURR_ALU_OUT, 5)  # K[j] handoff read
        dp[1].enable_delay_from_src(DelayInp.PREV_ALU_OUT, 0)  # a = v/2
        dp[1].pass_through_delay(1, 3, 4)
        dp[2].enable_alu(AluOp.MULTIPLY, AluInp.PREV_ALU_OUT, AluInp.PREV_DELAY_3)
        dp[2].pass_through_delay(0, 1, 4, 5)
        dp[3].enable_alu(AluOp.SUBTRACT, AluInp.PREV_DELAY_0, AluInp.PREV_DELAY_4)
        dp[3].enable_delay_from_src(DelayInp.PREV_ALU_OUT, 0)  # b = vv/12
        dp[3].pass_through_delay(1, 5)
        dp[4].enable_alu(AluOp.SUBTRACT, AluInp.PREV_ALU_OUT, AluInp.PREV_DELAY_0)
        dp[4].pass_through_delay(1, 5)
        dp[5].enable_alu(AluOp.MULTIPLY, AluInp.PREV_ALU_OUT, AluInp.PREV_DELAY_5)
        dp[5].pass_through_delay(1)
        dp[6].enable_alu(AluOp.ADD, AluInp.PREV_ALU_OUT, AluInp.PREV_DELAY_1)
        dp[7].enable_alu(AluOp.ADD, AluInp.PREV_ALU_OUT, AluInp.CURR_ALU_OUT)
        return u

    def _mk_even():
        # odd stream elements (2j+1): c1_j, K handoff, acc = u * c1; emit
        u = _inputs(UopConfig())
        u.trigger = (Trigger.SRC_TENSOR_DONE, Trigger.COUNT, Trigger.NONE)
        u.next_uop = (0, 1, 0)
        dp = u.datapath_config
        dp[0].enable_alu(AluOp.MULTIPLY, AluInp.PREV_ALU_OUT, AluInp.PREV_DELAY_0)
        dp[0].pass_through_delay(0, 1, 2, 3, 4)
        dp[1].enable_alu(AluOp.BYPASS, AluInp.PREV_DELAY_1)   # flop1 := K[j+1]
        dp[1].enable_delay_from_src(DelayInp.PREV_ALU_OUT, 5)  # vv
        dp[1].pass_through_delay(0, 2, 3, 4)
        dp[2].enable_alu(AluOp.MULTIPLY, AluInp.PREV_DELAY_0, AluInp.PREV_DELAY_2)
        dp[2].pass_through_delay(3, 4, 5)
        dp[3].enable_alu(AluOp.MULTIPLY, AluInp.PREV_DELAY_5, AluInp.PREV_DELAY_3)
        dp[3].enable_delay_from_src(DelayInp.PREV_ALU_OUT, 0)  # a = v/2
        dp[3].pass_through_delay(4)
        dp[4].enable_alu(AluOp.ADD, AluInp.PREV_DELAY_0, AluInp.PREV_DELAY_4)
        dp[4].enable_delay_from_src(DelayInp.PREV_ALU_OUT, 1)  # b = vv/12
        dp[5].enable_alu(AluOp.ADD, AluInp.PREV_ALU_OUT, AluInp.PREV_DELAY_1)
        dp[6].pass_through_alu()
        dp[7].enable_alu(AluOp.MULTIPLY, AluInp.CURR_ALU_OUT, AluInp.PREV_ALU_OUT)
        u.enable_output(OutSel.ALU_OUT, OutPath.WR0_LO)
        return u

    def _ref(in0, in1, s0, s1, imm2):
        p = in0.shape[0]
        n = in1.reshape(p, -1, 2).shape[1]
        v = in0.reshape(p, n, 2)[:, :, 0].astype(np.float64)
        kk = in1.reshape(p, n, 2)[:, :, 0].astype(np.float64)
        c1 = 1.0 + v * s0 + v * v * s1
        gam = -1.0 + v * s0 - v * v * s1
        out = np.empty((p, n), np.float64)
        acc = np.ones(p)
        kprev = np.ones(p)
        for j in range(n):
            acc = c1[:, j] * (acc + kk[:, j] + gam[:, j] * kprev)
            kprev = kk[:, j]
            out[:, j] = acc
        return out.astype(np.float32)

    class _HandDveOp2(DveOp):
        def compile(self, ver):
            spec = DveOpSpec(
                name=self.name,
                opcode=dve_ops.get_dve_sub_opcode(self.name),
                uops=[_mk_init(), _mk_odd(), _mk_even()],
                rd1_en=True,
            )
            spec.validate(ver)
            return spec

    op = _HandDveOp2(
        name="VF_PAIR_SCAN_ANT",
        spec=Spec(body=Src0 * Src1 * C0 * C1, reference=_ref),
        subdim=False,
        uops_sha={},
    )
    dve_ops._SUB_OPCODE_FOR_NAME[op.name] = 1 + len(OPS)
    OPS.append(op)
    dve_ops.CUSTOM_DVE_SPECS[op.name] = op.spec
    return get_vf_scan_op()

# (len, start) chunks of coarse rows J: producers run chunk-by-chunk so the
# PDE row loop can start after the first chunk.
JCH = [(1, 0), (1, 1), (2, 2), (3, 4), (4, 7), (6, 11), (8, 17), (8, 25),
       (8, 33), (8, 41), (8, 49), (6, 57)]


def _build_program():
    from contextlib import ExitStack

    import concourse.tile as tile
    from concourse import bacc, mybir

    DBL = get_dbl_op()
    f32 = mybir.dt.float32
    bf16 = mybir.dt.bfloat16
    Act = mybir.ActivationFunctionType

    nc = bacc.Bacc(
        "TRN2",
        target_bir_lowering=False,
        debug=False,
        enable_asserts=True,
        num_devices=8,
    )
    dyh_d = nc.dram_tensor("dyh", [32, 63 * 32], bf16, kind="ExternalInput").ap()
    dxh_d = nc.dram_tensor("dxh", [32, 4 * 63], bf16, kind="ExternalInput").ap()
    out_d = nc.dram_tensor("out", [4, 32], f32, kind="ExternalOutput").ap()

    with ExitStack() as ctx:
        tc = ctx.enter_context(tile.TileContext(nc))
        ws = ctx.enter_context(tc.tile_pool(name="ws", bufs=1))
        pp = ctx.enter_context(tc.tile_pool(name="pp", bufs=1, space="PSUM"))

        # Per-quadrant copies: group g's dy replica and dx slice live at
        # SBUF partitions 32g..32g+16 so the four matmul streams run as
        # independent diagonal PE tiles (tile_position (32g, 32g)).
        dxh_v = dxh_d.rearrange("d (g i) -> d g i", g=4)
        dyh_v = dyh_d.rearrange("d (j b) -> d j b", j=63)
        dxh_sb = ws.tile([128, 63], bf16)
        dyh_sb = ws.tile([128, 63, 32], bf16)
        # Distribute dma_start issue across engine queues: a DIRECT2D
        # issue costs ~600 ns on its sequencer, so 24 serial issues on Sync
        # would add ~14 us of head latency. Two waves: j 0:8 first so the
        # first matmul chunk's data lands fast.
        engs = [nc.sync, nc.scalar, nc.gpsimd]
        _ei = [0]

        def dma(out, in_):
            engs[_ei[0] % len(engs)].dma_start(out=out, in_=in_)
            _ei[0] += 1

        JW = 8
        for g in range(4):
            qs = slice(32 * g, 32 * g + 32)
            dma(dyh_sb[qs, 0:JW, :], dyh_v[:, 0:JW, :])
            dma(dxh_sb[qs, :], dxh_v[:, g, :])
        for g in range(4):
            qs = slice(32 * g, 32 * g + 32)
            dma(dyh_sb[qs, JW:63, :], dyh_v[:, JW:63, :])

        # K-row ping-pong buffers; 1.0 everywhere gives both the j=0 boundary
        # column and the i=0 boundary row (row 0 of K is all ones).
        kbuf = ws.tile([128, 2, 128], f32)
        nc.vector.memset(kbuf[:, :, :], 1.0)

        ps = pp.tile([128, 63, 64], f32)  # coarse inc; row J at [:, J, 0:63]
        # ccc: coarse interleaved [gamma_J, c1_J] pairs per row (stride-2
        # writes from the poly ops). cc3: fine stream [gamma_j, c1_j, c1_j]
        # per fine column (Act-expanded; the double-row op's SRC_0).
        ccc = ws.tile([128, 63, 63], f32)
        cc3 = ws.tile([128, 63, 63, 2, 3], f32)
        C1P, GMP = get_poly_ops()

        for ln, st in JCH:
            jsl = slice(st, st + ln)
            for j in range(st, st + ln):
                for g in range(4):
                    po = slice(32 * g, 32 * g + 32)
                    qs = slice(32 * g, 32 * g + 32)
                    tp = (32 * g, 32 * g)
                    # K=32 contraction-concatenation: [dyh; dyh] x [dxh; dxl]
                    # = dyh*dxh + dyh*dxl in ONE matmul (half the ldweights).
                    nc.tensor.matmul(
                        ps[po, j, 0:63], dyh_sb[qs, j, :], dxh_sb[qs, :],
                        start=True, stop=True, tile_position=tp,
                    )
            nc.vector._custom_dve(
                GMP, out=ccc[:, jsl, :], in0=ps[:, jsl, 0:63],
                s0=0.5, s1=1.0 / 12.0, imm2=0.0,
            )
            # Act expansion to the fine [gamma, v, *] stream (the row op
            # derives c1 = v - gamma inline): gamma -> (J, d, 0), v ->
            # (J, d, 1). Slot 2 is never read by any ALU so it stays
            # unwritten. v reads PSUM directly (3-free-dim APs only).
            gsrc = ccc[:, jsl, :].unsqueeze(3).broadcast_to((128, ln, 63, 2))
            vsrc = ps[:, jsl, 0:63].unsqueeze(3).broadcast_to((128, ln, 63, 2))
            nc.scalar.activation(
                out=cc3[:, jsl, :, :, 0], in_=gsrc,
                func=Act.Copy, bias=0.0, scale=1.0,
            )
            nc.scalar.activation(
                out=cc3[:, jsl, :, :, 1], in_=vsrc,
                func=Act.Copy, bias=0.0, scale=1.0,
            )
            # Interleave the row-pair instructions with the producers: the
            # DVE queue is in-order, so rows of this chunk issue right after
            # its producers and absorb later chunks' PE/DMA latency.
            for ri in range(st, st + ln):
                srcb = kbuf[:, ri & 1, :]
                dstb = kbuf[:, 1 - (ri & 1), :]
                win = srcb[:, 1:127].unsqueeze(2).broadcast_to((128, 126, 3))
                cin = cc3[:, ri, :, :, :].rearrange("p j d s -> p (j d s)")
                nc.vector._custom_dve(
                    DBL, out=dstb[:, 1:127], in0=cin, in1=win,
                    s0=0.0, s1=0.0, imm2=0.0,
                )

        # Compact the result column K[*, 126] (in kbuf[:, 0, :] after row 125)
        # into 4 partitions via a 32x32 block transpose, then one 4-descriptor
        # DMA: tt[32B + i, f] = kbuf[32B + f, 0, 96 + i] -> row i=30 holds col
        # 126 for the 32 pairs of partition-block B.
        # Compact result column K[*, 126] via 32x32 block transpose, then
        # four 1-descriptor DMAs (vs 128 4-byte descriptors, ~7 us).
        tt = ws.tile([128, 32], f32)
        nc.vector.transpose(tt[:], kbuf[:, 1, 96:128])
        oeng = [nc.scalar, nc.gpsimd, nc.sync, nc.scalar]
        for g in range(4):
            oeng[g].dma_start(
                out=out_d[g : g + 1, :], in_=tt[32 * g + 30 : 32 * g + 31, :]
            )

    nc.compile()
    return nc


def _get_nc():
    if "nc" not in _STATE:
        _STATE["nc"] = _build_program()
    return _STATE["nc"]


def _make_inputs(xs: np.ndarray, ys: np.ndarray):
    xs = np.asarray(xs, dtype=np.float32)
    ys = np.asarray(ys, dtype=np.float32)
    dxs_all = (xs[:, 1:, :] - xs[:, :-1, :]) * np.float32(0.25)  # (32, 63, 16)
    dys = ys[:, 1:, :] - ys[:, :-1, :]                           # (32, 63, 16)

    import ml_dtypes

    bf16 = ml_dtypes.bfloat16
    dysT = np.ascontiguousarray(dys.transpose(2, 1, 0))          # [d, j, b]
    dyh1 = dysT.astype(bf16).reshape(16, 63 * 32)
    dyh = np.ascontiguousarray(np.concatenate([dyh1, dyh1], axis=0))

    in_maps = []
    for c in range(8):
        dxs_c = np.ascontiguousarray(
            dxs_all[4 * c : 4 * c + 4].transpose(2, 0, 1).reshape(16, 4 * 63)
        )  # [d, (a', i)]
        dxhh = dxs_c.astype(bf16)
        dxll = (dxs_c - dxhh.astype(np.float32)).astype(bf16)
        dxh = np.ascontiguousarray(np.concatenate([dxhh, dxll], axis=0))
        in_maps.append({"dyh": dyh, "dxh": dxh})
    return in_maps


def _run(nc, in_maps, **kwargs):
    from concourse.bass_utils import run_bass_kernel_spmd

    return run_bass_kernel_spmd(nc, in_maps, list(range(8)), **kwargs)


def kernel(xs: np.ndarray, ys: np.ndarray) -> np.ndarray:
    nc = _get_nc()
    in_maps = _make_inputs(xs, ys)
    res = _run(nc, in_maps)
    out = np.concatenate(
        [np.asarray(res.results[c]["out"]).reshape(4, 32) for c in range(8)], axis=0
    )
    return out.astype(np.float32)


# revision 29
# speedup vs baseline: 2399.8563x; 1.0278x over previous
"""Trainium2 Bass kernel for the signature-kernel (Goursat PDE) problem.

Full inputs: xs (32, 64, 16) f32, ys (32, 64, 16) f32.
Output: (32, 32) f32 signature-kernel Gram matrix.

Strategy (8 NeuronCores, SPMD, no collectives):
  - Shard batch_x across cores: core c owns a in {4c..4c+3} -> 4*32 = 128
    (x, y) pairs, one pair per SBUF partition.
  - Coarse increments inc[a,b,I,J] = sum_d Dxs[a,I,d] Dys[b,J,d] / 4 are
    computed on-device with PE matmuls from DENSE hi/lo bf16 splits: per
    coarse row J and per a-group g, out partitions 32g..32g+32 contract
    lhsT = DysT[d, J, b] (16x32) against rhs = Dxs_g[d, I] (16x63); three
    accumulating matmuls (hi*hi + hi*lo + lo*hi) give fp32-level accuracy.
  - The Goursat PDE recurrence K[i+1,j+1] = c1*(K[i+1,j] + K[i,j+1])
    - c2*K[i,j] with the dyadically-refined (2x-duplicated) coefficients
    c1 = 1 + v/2 + v^2/12, c2 = 1 - v^2/12 is solved with ONE custom DVE
    instruction per ROW PAIR (63 instructions, get_dbl_op): dyadic
    refinement duplicates rows, so rows 2I and 2I+1 share coefficients
    and a hand-written 4-uop program advances BOTH in one pass at 1
    stream element/cycle -- u' = c1*(u + K0[j+1] + gamma*K0[j]),
    v' = c1*(v + u' + gamma*u) -- emitting only v (the intermediate row
    never materializes). gamma = -c2/c1 = -1 + v/2 - v^2/12 + O(v^4).
    The v accumulator crosses elements through slice 7's operand flop
    (alu_out_a_enable on the O element, NEXT_ALU_OUT_A read on C).
    Coefficients stream as [gamma_j, c1_j, c1_j] per fine column (379
    elements, ~553 ns per double row), produced per chunk by two Part-I
    custom poly ops (PSUM vf -> interleaved coarse pairs) and expanded
    by the otherwise-idle Act engine.
  - The final column K[*, 126] is compacted with a 32x32 StreamTranspose
    so the output DMA is 4 descriptors instead of 128.
"""

import os
import sys

import numpy as np

for _p in ("/opt/trn_rl_repo", "/root/.axon_site", "/root/.axon_site/_ro/trn_rl_repo",
           "/root/.axon_site/_ro/pypackages"):
    if os.path.isdir(_p) and _p not in sys.path:
        sys.path.append(_p)

_STATE: dict = {}
_OP2 = None

_OPS_CACHE: dict = {}


def _register_hand_op(name, make_uops, rd1, ref, spec_body):
    """Register a hand-written custom DVE op; returns the DveOp."""
    import concourse.dve_ops as dve_ops
    from concourse.dve_ops import DveOp, OPS
    from concourse.dve_spec import Spec
    from concourse.dve_uop import DveOpSpec

    for op in OPS:
        if op.name == name:
            return op

    class _H(DveOp):
        def compile(self, ver):
            spec = DveOpSpec(
                name=self.name,
                opcode=dve_ops.get_dve_sub_opcode(self.name),
                uops=make_uops(),
                rd1_en=rd1,
            )
            spec.validate(ver)
            return spec

    op = _H(name=name, spec=Spec(body=spec_body, reference=ref),
            subdim=False, uops_sha={})
    dve_ops._SUB_OPCODE_FOR_NAME[op.name] = 1 + len(OPS)
    OPS.append(op)
    dve_ops.CUSTOM_DVE_SPECS[op.name] = op.spec
    return op


def _register_spec_op(name, body, ref):
    """Register a Part-I Spec op (auto-lowered), bypassing the sha pin."""
    import concourse.dve_ops as dve_ops
    from concourse.dve_ops import DveOp, OPS
    from concourse.dve_spec import Spec, lower, _has_src1
    from concourse.dve_uop import DveOpSpec

    for op in OPS:
        if op.name == name:
            return op

    class _S(DveOp):
        def compile(self, ver):
            spec = DveOpSpec(
                name=self.name,
                opcode=dve_ops.get_dve_sub_opcode(self.name),
                uops=lower(self.spec, ver=ver),
                rd1_en=_has_src1(self.spec),
            )
            spec.validate(ver)
            return spec

    op = _S(name=name, spec=Spec(body=body, reference=ref),
            subdim=False, uops_sha={})
    dve_ops._SUB_OPCODE_FOR_NAME[op.name] = 1 + len(OPS)
    OPS.append(op)
    dve_ops.CUSTOM_DVE_SPECS[op.name] = op.spec
    return op


def get_poly_ops():
    """c1 = 1 + v*s0 + v^2*s1 ; gamma = -1 + v*s0 - v^2*s1 (= -c2/c1 +O(v^4))."""
    from concourse.dve_spec import Src0, C0, C1, One, sq

    c1 = _register_spec_op(
        "C1_POLY_ANT",
        Src0 * C0 + sq(Src0) * C1 + One,
        lambda in0, in1, s0, s1, imm2: (
            1.0 + in0.astype("float64") * s0 + in0.astype("float64") ** 2 * s1
        ).astype("float32"),
    )
    gm = _register_spec_op(
        "GAMMA_POLY_ANT",
        Src0 * C0 - sq(Src0) * C1 - One,
        lambda in0, in1, s0, s1, imm2: (
            in0.astype("float64") * s0 - in0.astype("float64") ** 2 * s1 - 1.0
        ).astype("float32"),
    )
    return c1, gm


def get_dbl_op():
    """Two PDE rows per instruction; see exp_custom3.py for the derivation.

    Stream (3 elements per fine column j): SRC_0 = [gamma_j, c1_j, c1_j],
    SRC_1 = K0[j+1] (x3). Per pair: u' = c1*(u + K0[j+1] + gamma*K0[j]),
    v' = c1*(v + u' + gamma*u); only v' is emitted (the intermediate row u
    never materializes). v is handed across elements via slice 7's a-flop
    (alu_out_a_enable on O, NEXT_ALU_OUT_A read on C at slice 6).
    """
    from concourse.dve_uop import (
        UopConfig, AluOp, AluInp, InpSel, OutSel, OutPath, Trigger, DelayInp,
    )
    import numpy as np

    def mk_init():
        u = UopConfig()
        u.enable_input(InpSel.ONE_F32, 1)
        u.repeat_count = 1
        u.trigger = (Trigger.COUNT, Trigger.NONE, Trigger.NONE)
        u.next_uop = (1, 0, 0)
        dp = u.datapath_config
        dp[0].enable_alu(AluOp.BYPASS, AluInp.PREV_DELAY_0)
        for b in range(7):
            dp[b].pass_through_delay(0)
        dp[5].enable_alu(AluOp.BYPASS, AluInp.PREV_DELAY_0)
        dp[7].enable_alu(AluOp.BYPASS, AluInp.PREV_DELAY_0)
        dp[7].alu_out_a_enable = 1
        return u

    def mk_O():
        u = UopConfig()
        u.enable_input(InpSel.SRC_0, 1)
        u.enable_input(InpSel.SRC_1, 2)
        u.require_inp0 = u.require_inp1 = 1
        u.repeat_count = 1
        u.trigger = (Trigger.SRC_TENSOR_DONE, Trigger.COUNT, Trigger.NONE)
        u.next_uop = (0, 2, 0)
        dp = u.datapath_config
        dp[0].enable_alu(AluOp.MULTIPLY, AluInp.PREV_DELAY_0, AluInp.CURR_ALU_OUT)
        dp[0].pass_through_delay(0, 1)
        dp[1].enable_alu(AluOp.ADD, AluInp.PREV_ALU_OUT, AluInp.PREV_DELAY_1)
        dp[1].pass_through_delay(0)
        dp[2].enable_delay_from_src(DelayInp.PREV_ALU_OUT, 1)
        dp[2].pass_through_delay(0)
        dp[3].enable_alu(AluOp.BYPASS, AluInp.PREV_DELAY_0)  # flop3 := gamma_j
        dp[3].pass_through_delay(0, 1)
        dp[4].pass_through_delay(0, 1)
        dp[5].enable_alu(AluOp.ADD, AluInp.PREV_DELAY_1, AluInp.CURR_ALU_OUT)
        dp[5].enable_delay_from_src(DelayInp.CURR_ALU_OUT, 2)
        dp[5].pass_through_delay(0)
        dp[6].enable_alu(AluOp.MULTIPLY, AluInp.PREV_DELAY_0, AluInp.PREV_DELAY_2)
        dp[7].enable_alu(AluOp.BYPASS, AluInp.CURR_ALU_OUT, AluInp.CURR_ALU_OUT)
        dp[7].alu_out_a_enable = 1
        return u

    def mk_E():
        u = UopConfig()
        u.enable_input(InpSel.SRC_0, 1)
        u.enable_input(InpSel.SRC_1, 2)
        u.require_inp0 = u.require_inp1 = 1
        u.repeat_count = 1
        u.trigger = (Trigger.SRC_TENSOR_DONE, Trigger.COUNT, Trigger.NONE)
        u.next_uop = (0, 3, 0)
        dp = u.datapath_config
        dp[0].enable_alu(AluOp.BYPASS, AluInp.PREV_DELAY_1)
        for b in range(3):
            dp[b].pass_through_delay(0)
        # s3: c1 = v - gamma (gamma parked in flop3 by O)
        dp[3].enable_alu(AluOp.SUBTRACT, AluInp.PREV_DELAY_0, AluInp.CURR_ALU_OUT)
        dp[4].pass_through_alu()  # carry c1
        dp[5].enable_alu(AluOp.MULTIPLY, AluInp.CURR_ALU_OUT, AluInp.PREV_ALU_OUT)
        dp[6].enable_alu(AluOp.ADD, AluInp.CURR_ALU_OUT, AluInp.PREV_ALU_OUT)
        return u

    def mk_C():
        u = UopConfig()
        u.enable_input(InpSel.SRC_0, 1)
        u.enable_input(InpSel.SRC_1, 2)
        u.require_inp0 = u.require_inp1 = 1
        u.repeat_count = 1
        u.trigger = (Trigger.SRC_TENSOR_DONE, Trigger.COUNT, Trigger.NONE)
        u.next_uop = (0, 1, 0)
        dp = u.datapath_config
        for b in range(6):
            dp[b].pass_through_delay(0)
        # s3: pick up c1 (flop3, written by E one cycle earlier) into lane1
        dp[3].enable_delay_from_src(DelayInp.CURR_ALU_OUT, 1)
        dp[4].pass_through_delay(1)
        dp[5].pass_through_delay(1)
        dp[6].enable_alu(AluOp.ADD, AluInp.CURR_ALU_OUT, AluInp.NEXT_ALU_OUT_A)
        dp[6].pass_through_delay(0, 1)
        dp[7].enable_alu(AluOp.MULTIPLY, AluInp.PREV_ALU_OUT, AluInp.PREV_DELAY_1)
        u.enable_output(OutSel.ALU_OUT, OutPath.WR0_LO)
        return u

    def ref(in0, in1, s0, s1, imm2):
        p = in0.shape[0]
        n = in0.shape[-1] // 3
        cc = in0.reshape(p, n, 3).astype(np.float64)
        kk = in1.reshape(p, n, 3)[:, :, 0].astype(np.float64)
        gam = cc[:, :, 0]
        c1 = cc[:, :, 1] - gam  # slot 1 carries v; c1 = v - gamma
        out = np.empty((p, n), np.float64)
        u = np.ones(p); v = np.ones(p); kprev = np.ones(p)
        for j in range(n):
            un = c1[:, j] * (u + kk[:, j] + gam[:, j] * kprev)
            v = c1[:, j] * (v + un + gam[:, j] * u)
            u, kprev = un, kk[:, j]
            out[:, j] = v
        return out.astype(np.float32)

    from concourse.dve_spec import Src0, Src1

    return _register_hand_op(
        "DBL_PAIR_SCAN_ANT", lambda: [mk_init(), mk_O(), mk_E(), mk_C()],
        True, ref, Src0 * Src1,
    )




def get_vf_scan_op():
    """Custom DVE op VF_PAIR_SCAN_ANT (hand-written 3-uop program).

    Reads the coarse vf value v_j (duplicated 4x via a stride-0 AP) and the
    previous K row (K[j+1] duplicated 2x), computes c1/gamma inline:
        c1 = 1 + v/2 + v^2/12,  gamma = -1 + v/2 - v^2/12  (= -c2/c1 + O(v^4))
        acc_j = c1_j * (acc_{j-1} + K[j+1] + gamma_j * K[j])
    and emits acc_j (= K_new[j+1]) on every second stream element, at one
    stream element per cycle. s0 = 0.5 (CONST_0), s1 = 1/12 (CONST_1).
    """
    global _OP2
    if _OP2 is not None:
        return _OP2
    import concourse.dve_ops as dve_ops
    from concourse.dve_ops import DveOp, OPS
    from concourse.dve_spec import Spec, Src0, Src1, C0, C1
    from concourse.dve_uop import (
        DveOpSpec,
        UopConfig,
        AluOp,
        AluInp,
        InpSel,
        OutSel,
        OutPath,
        Trigger,
        DelayInp,
    )

    for op in OPS:
        if op.name == "VF_PAIR_SCAN_ANT":
            _OP2 = op
            return op

    def _inputs(u):
        u.enable_input(InpSel.SRC_0, 0)    # v -> blk0 PREV_ALU_OUT
        u.enable_input(InpSel.SRC_0, 1)    # v -> lane 0
        u.enable_input(InpSel.SRC_1, 2)    # K[j+1] -> lane 1
        u.enable_input(InpSel.CONST_0, 3)  # 0.5 -> lane 2
        u.enable_input(InpSel.CONST_1, 4)  # 1/12 -> lane 3
        u.enable_input(InpSel.ONE_F32, 5)  # 1.0 -> lane 4
        u.require_inp0 = u.require_inp1 = 1
        u.repeat_count = 1
        return u

    def _mk_init():
        u = UopConfig()
        u.enable_input(InpSel.ONE_F32, 1)  # lane 0
        u.repeat_count = 1
        u.trigger = (Trigger.COUNT, Trigger.NONE, Trigger.NONE)
        u.next_uop = (1, 0, 0)
        dp = u.datapath_config
        dp[0].pass_through_delay(0)
        dp[1].enable_alu(AluOp.BYPASS, AluInp.PREV_DELAY_0)  # flop1 := 1.0 (K[0])
        for b in (1, 2, 3, 4, 5, 6):
            dp[b].pass_through_delay(0)
        dp[7].enable_alu(AluOp.BYPASS, AluInp.PREV_DELAY_0)  # flop7 := 1.0 (acc)
        return u

    def _mk_odd():
        # even stream elements (2j): compute gamma_j, p, m, u
        u = _inputs(UopConfig())
        u.trigger = (Trigger.SRC_TENSOR_DONE, Trigger.COUNT, Trigger.NONE)
        u.next_uop = (0, 2, 0)
        dp = u.datapath_config
        dp[0].enable_alu(AluOp.MULTIPLY, AluInp.PREV_ALU_OUT, AluInp.PREV_DELAY_2)
        dp[0].pass_through_delay(0, 1, 3, 4)
        dp[1].enable_alu(AluOp.MULTIPLY, AluInp.PREV_DELAY_0, AluInp.PREV_DELAY_0)
        dp[1].enable_delay_from_src(DelayInp.CURR_ALU_OUT, 5)  # K[j] handoff read
        dp[1].enable_delay_from_src(DelayInp.PREV_ALU_OUT, 0)  # a = v/2
        dp[1].pass_through_delay(1, 3, 4)
        dp[2].enable_alu(AluOp.MULTIPLY, AluInp.PREV_ALU_OUT, AluInp.PREV_DELAY_3)
        dp[2].pass_through_delay(0, 1, 4, 5)
        dp[3].enable_alu(AluOp.SUBTRACT, AluInp.PREV_DELAY_0, AluInp.PREV_DELAY_4)
        dp[3].enable_delay_from_src(DelayInp.PREV_ALU_OUT, 0)  # b = vv/12
        dp[3].pass_through_delay(1, 5)
        dp[4].enable_alu(AluOp.SUBTRACT, AluInp.PREV_ALU_OUT, AluInp.PREV_DELAY_0)
        dp[4].pass_through_delay(1, 5)
        dp[5].enable_alu(AluOp.MULTIPLY, AluInp.PREV_ALU_OUT, AluInp.PREV_DELAY_5)
        dp[5].pass_through_delay(1)
        dp[6].enable_alu(AluOp.ADD, AluInp.PREV_ALU_OUT, AluInp.PREV_DELAY_1)
        dp[7].enable_alu(AluOp.ADD, AluInp.PREV_ALU_OUT, AluInp.CURR_ALU_OUT)
        return u

    def _mk_even():
        # odd stream elements (2j+1): c1_j, K handoff, acc = u * c1; emit
        u = _inputs(UopConfig())
        u.trigger = (Trigger.SRC_TENSOR_DONE, Trigger.COUNT, Trigger.NONE)
        u.next_uop = (0, 1, 0)
        dp = u.datapath_config
        dp[0].enable_alu(AluOp.MULTIPLY, AluInp.PREV_ALU_OUT, AluInp.PREV_DELAY_0)
        dp[0].pass_through_delay(0, 1, 2, 3, 4)
        dp[1].enable_alu(AluOp.BYPASS, AluInp.PREV_DELAY_1)   # flop1 := K[j+1]
        dp[1].enable_delay_from_src(DelayInp.PREV_ALU_OUT, 5)  # vv
        dp[1].pass_through_delay(0, 2, 3, 4)
        dp[2].enable_alu(AluOp.MULTIPLY, AluInp.PREV_DELAY_0, AluInp.PREV_DELAY_2)
        dp[2].pass_through_delay(3, 4, 5)
        dp[3].enable_alu(AluOp.MULTIPLY, AluInp.PREV_DELAY_5, AluInp.PREV_DELAY_3)
        dp[3].enable_delay_from_src(DelayInp.PREV_ALU_OUT, 0)  # a = v/2
        dp[3].pass_through_delay(4)
        dp[4].enable_alu(AluOp.ADD, AluInp.PREV_DELAY_0, AluInp.PREV_DELAY_4)
        dp[4].enable_delay_from_src(DelayInp.PREV_ALU_OUT, 1)  # b = vv/12
        dp[5].enable_alu(AluOp.ADD, AluInp.PREV_ALU_OUT, AluInp.PREV_DELAY_1)
        dp[6].pass_through_alu()
        dp[7].enable_alu(AluOp.MULTIPLY, AluInp.CURR_ALU_OUT, AluInp.PREV_ALU_OUT)
        u.enable_output(OutSel.ALU_OUT, OutPath.WR0_LO)
        return u

    def _ref(in0, in1, s0, s1, imm2):
        p = in0.shape[0]
        n = in1.reshape(p, -1, 2).shape[1]
        v = in0.reshape(p, n, 2)[:, :, 0].astype(np.float64)
        kk = in1.reshape(p, n, 2)[:, :, 0].astype(np.float64)
        c1 = 1.0 + v * s0 + v * v * s1
        gam = -1.0 + v * s0 - v * v * s1
        out = np.empty((p, n), np.float64)
        acc = np.ones(p)
        kprev = np.ones(p)
        for j in range(n):
            acc = c1[:, j] * (acc + kk[:, j] + gam[:, j] * kprev)
            kprev = kk[:, j]
            out[:, j] = acc
        return out.astype(np.float32)

    class _HandDveOp2(DveOp):
        def compile(self, ver):
            spec = DveOpSpec(
                name=self.name,
                opcode=dve_ops.get_dve_sub_opcode(self.name),
                uops=[_mk_init(), _mk_odd(), _mk_even()],
                rd1_en=True,
            )
            spec.validate(ver)
            return spec

    op = _HandDveOp2(
        name="VF_PAIR_SCAN_ANT",
        spec=Spec(body=Src0 * Src1 * C0 * C1, reference=_ref),
        subdim=False,
        uops_sha={},
    )
    dve_ops._SUB_OPCODE_FOR_NAME[op.name] = 1 + len(OPS)
    OPS.append(op)
    dve_ops.CUSTOM_DVE_SPECS[op.name] = op.spec
    return get_vf_scan_op()

# (len, start) chunks of coarse rows J: producers run chunk-by-chunk so the
# PDE row loop can start after the first chunk.
JCH = [(1, 0), (1, 1), (2, 2), (3, 4), (4, 7), (6, 11), (8, 17), (8, 25),
       (8, 33), (8, 41), (8, 49), (6, 57)]


def _build_program():
    from contextlib import ExitStack

    import concourse.tile as tile
    from concourse import bacc, mybir

    DBL = get_dbl_op()
    f32 = mybir.dt.float32
    bf16 = mybir.dt.bfloat16
    Act = mybir.ActivationFunctionType

    nc = bacc.Bacc(
        "TRN2",
        target_bir_lowering=False,
        debug=False,
        enable_asserts=True,
        num_devices=8,
    )
    dyh_d = nc.dram_tensor("dyh", [32, 63 * 32], bf16, kind="ExternalInput").ap()
    dxh_d = nc.dram_tensor("dxh", [32, 4 * 63], bf16, kind="ExternalInput").ap()
    out_d = nc.dram_tensor("out", [4, 32], f32, kind="ExternalOutput").ap()

    with ExitStack() as ctx:
        tc = ctx.enter_context(tile.TileContext(nc))
        ws = ctx.enter_context(tc.tile_pool(name="ws", bufs=1))
        pp = ctx.enter_context(tc.tile_pool(name="pp", bufs=1, space="PSUM"))

        # Per-quadrant copies: group g's dy replica and dx slice live at
        # SBUF partitions 32g..32g+16 so the four matmul streams run as
        # independent diagonal PE tiles (tile_position (32g, 32g)).
        dxh_v = dxh_d.rearrange("d (g i) -> d g i", g=4)
        dyh_v = dyh_d.rearrange("d (j b) -> d j b", j=63)
        dxh_sb = ws.tile([128, 63], bf16)
        dyh_sb = ws.tile([128, 63, 32], bf16)
        # Distribute dma_start issue across engine queues: a DIRECT2D
        # issue costs ~600 ns on its sequencer, so 24 serial issues on Sync
        # would add ~14 us of head latency. Two waves: j 0:8 first so the
        # first matmul chunk's data lands fast.
        engs = [nc.sync, nc.scalar, nc.gpsimd]
        _ei = [0]

        def dma(out, in_):
            engs[_ei[0] % len(engs)].dma_start(out=out, in_=in_)
            _ei[0] += 1

        JW = 8
        for g in range(4):
            qs = slice(32 * g, 32 * g + 32)
            dma(dyh_sb[qs, 0:JW, :], dyh_v[:, 0:JW, :])
            dma(dxh_sb[qs, :], dxh_v[:, g, :])
        for g in range(4):
            qs = slice(32 * g, 32 * g + 32)
            dma(dyh_sb[qs, JW:63, :], dyh_v[:, JW:63, :])

        # K-row ping-pong buffers; 1.0 everywhere gives both the j=0 boundary
        # column and the i=0 boundary row (row 0 of K is all ones).
        kbuf = ws.tile([128, 2, 128], f32)
        nc.vector.memset(kbuf[:, :, :], 1.0)

        ps = pp.tile([128, 63, 64], f32)  # coarse inc; row J at [:, J, 0:63]
        # ccc: coarse interleaved [gamma_J, c1_J] pairs per row (stride-2
        # writes from the poly ops). cc3: fine stream [gamma_j, c1_j, c1_j]
        # per fine column (Act-expanded; the double-row op's SRC_0).
        ccc = ws.tile([128, 63, 63], f32)
        cc3 = ws.tile([128, 63, 63, 2, 3], f32)
        C1P, GMP = get_poly_ops()

        def produce(ln, st):
            jsl = slice(st, st + ln)
            for j in range(st, st + ln):
                for g in range(4):
                    po = slice(32 * g, 32 * g + 32)
                    qs = slice(32 * g, 32 * g + 32)
                    tp = (32 * g, 32 * g)
                    # K=32 contraction-concatenation: [dyh; dyh] x [dxh; dxl]
                    # = dyh*dxh + dyh*dxl in ONE matmul (half the ldweights).
                    nc.tensor.matmul(
                        ps[po, j, 0:63], dyh_sb[qs, j, :], dxh_sb[qs, :],
                        start=True, stop=True, tile_position=tp,
                    )
            nc.vector._custom_dve(
                GMP, out=ccc[:, jsl, :], in0=ps[:, jsl, 0:63],
                s0=0.5, s1=1.0 / 12.0, imm2=0.0,
            )
            # Act expansion to the fine [gamma, v, *] stream (the row op
            # derives c1 = v - gamma inline): gamma -> (J, d, 0), v ->
            # (J, d, 1). Slot 2 is never read by any ALU so it stays
            # unwritten. v reads PSUM directly (3-free-dim APs only).
            gsrc = ccc[:, jsl, :].unsqueeze(3).broadcast_to((128, ln, 63, 2))
            vsrc = ps[:, jsl, 0:63].unsqueeze(3).broadcast_to((128, ln, 63, 2))
            nc.scalar.activation(
                out=cc3[:, jsl, :, :, 0], in_=gsrc,
                func=Act.Copy, bias=0.0, scale=1.0,
            )
            nc.scalar.activation(
                out=cc3[:, jsl, :, :, 1], in_=vsrc,
                func=Act.Copy, bias=0.0, scale=1.0,
            )
        def rows(ln, st):
            for ri in range(st, st + ln):
                srcb = kbuf[:, ri & 1, :]
                dstb = kbuf[:, 1 - (ri & 1), :]
                win = srcb[:, 1:127].unsqueeze(2).broadcast_to((128, 126, 3))
                cin = cc3[:, ri, :, :, :].rearrange("p j d s -> p (j d s)")
                nc.vector._custom_dve(
                    DBL, out=dstb[:, 1:127], in0=cin, in1=win,
                    s0=0.0, s1=0.0, imm2=0.0,
                )

        # Software-pipelined issue with one-chunk lookahead: the DVE and Act
        # queues are in-order, so chunk c+1's producers are issued before
        # chunk c's rows; the Act expansion latency hides behind row
        # execution instead of stalling each chunk boundary.
        for ci in range(len(JCH) + 1):
            if ci < len(JCH):
                produce(*JCH[ci])
            if ci >= 1:
                rows(*JCH[ci - 1])

        # Compact the result column K[*, 126] (in kbuf[:, 0, :] after row 125)
        # into 4 partitions via a 32x32 block transpose, then one 4-descriptor
        # DMA: tt[32B + i, f] = kbuf[32B + f, 0, 96 + i] -> row i=30 holds col
        # 126 for the 32 pairs of partition-block B.
        # Compact result column K[*, 126] via 32x32 block transpose, then
        # four 1-descriptor DMAs (vs 128 4-byte descriptors, ~7 us).
        tt = ws.tile([128, 32], f32)
        nc.vector.transpose(tt[:], kbuf[:, 1, 96:128])
        oeng = [nc.scalar, nc.gpsimd, nc.sync, nc.scalar]
        for g in range(4):
            oeng[g].dma_start(
                out=out_d[g : g + 1, :], in_=tt[32 * g + 30 : 32 * g + 31, :]
            )

    nc.compile()
    return nc


def _get_nc():
    if "nc" not in _STATE:
        _STATE["nc"] = _build_program()
    return _STATE["nc"]


def _make_inputs(xs: np.ndarray, ys: np.ndarray):
    xs = np.asarray(xs, dtype=np.float32)
    ys = np.asarray(ys, dtype=np.float32)
    dxs_all = (xs[:, 1:, :] - xs[:, :-1, :]) * np.float32(0.25)  # (32, 63, 16)
    dys = ys[:, 1:, :] - ys[:, :-1, :]                           # (32, 63, 16)

    import ml_dtypes

    bf16 = ml_dtypes.bfloat16
    dysT = np.ascontiguousarray(dys.transpose(2, 1, 0))          # [d, j, b]
    dyh1 = dysT.astype(bf16).reshape(16, 63 * 32)
    dyh = np.ascontiguousarray(np.concatenate([dyh1, dyh1], axis=0))

    in_maps = []
    for c in range(8):
        dxs_c = np.ascontiguousarray(
            dxs_all[4 * c : 4 * c + 4].transpose(2, 0, 1).reshape(16, 4 * 63)
        )  # [d, (a', i)]
        dxhh = dxs_c.astype(bf16)
        dxll = (dxs_c - dxhh.astype(np.float32)).astype(bf16)
        dxh = np.ascontiguousarray(np.concatenate([dxhh, dxll], axis=0))
        in_maps.append({"dyh": dyh, "dxh": dxh})
    return in_maps


def _run(nc, in_maps, **kwargs):
    from concourse.bass_utils import run_bass_kernel_spmd

    return run_bass_kernel_spmd(nc, in_maps, list(range(8)), **kwargs)


def kernel(xs: np.ndarray, ys: np.ndarray) -> np.ndarray:
    nc = _get_nc()
    in_maps = _make_inputs(xs, ys)
    res = _run(nc, in_maps)
    out = np.concatenate(
        [np.asarray(res.results[c]["out"]).reshape(4, 32) for c in range(8)], axis=0
    )
    return out.astype(np.float32)


# revision 30
# speedup vs baseline: 2438.4879x; 1.0161x over previous
"""Trainium2 Bass kernel for the signature-kernel (Goursat PDE) problem.

Full inputs: xs (32, 64, 16) f32, ys (32, 64, 16) f32.
Output: (32, 32) f32 signature-kernel Gram matrix.

Strategy (8 NeuronCores, SPMD, no collectives):
  - Shard batch_x across cores: core c owns a in {4c..4c+3} -> 4*32 = 128
    (x, y) pairs, one pair per SBUF partition.
  - Coarse increments inc[a,b,I,J] = sum_d Dxs[a,I,d] Dys[b,J,d] / 4 are
    computed on-device with PE matmuls from DENSE hi/lo bf16 splits: per
    coarse row J and per a-group g, out partitions 32g..32g+32 contract
    lhsT = DysT[d, J, b] (16x32) against rhs = Dxs_g[d, I] (16x63); three
    accumulating matmuls (hi*hi + hi*lo + lo*hi) give fp32-level accuracy.
  - The Goursat PDE recurrence K[i+1,j+1] = c1*(K[i+1,j] + K[i,j+1])
    - c2*K[i,j] with the dyadically-refined (2x-duplicated) coefficients
    c1 = 1 + v/2 + v^2/12, c2 = 1 - v^2/12 is solved with ONE custom DVE
    instruction per ROW PAIR (63 instructions, get_dbl_op): dyadic
    refinement duplicates rows, so rows 2I and 2I+1 share coefficients
    and a hand-written 4-uop program advances BOTH in one pass at 1
    stream element/cycle -- u' = c1*(u + K0[j+1] + gamma*K0[j]),
    v' = c1*(v + u' + gamma*u) -- emitting only v (the intermediate row
    never materializes). gamma = -c2/c1 = -1 + v/2 - v^2/12 + O(v^4).
    The v accumulator crosses elements through slice 7's operand flop
    (alu_out_a_enable on the O element, NEXT_ALU_OUT_A read on C).
    Coefficients stream as [gamma_j, c1_j, c1_j] per fine column (379
    elements, ~553 ns per double row), produced per chunk by two Part-I
    custom poly ops (PSUM vf -> interleaved coarse pairs) and expanded
    by the otherwise-idle Act engine.
  - The final column K[*, 126] is compacted with a 32x32 StreamTranspose
    so the output DMA is 4 descriptors instead of 128.
"""

import os
import sys

import numpy as np

for _p in ("/opt/trn_rl_repo", "/root/.axon_site", "/root/.axon_site/_ro/trn_rl_repo",
           "/root/.axon_site/_ro/pypackages"):
    if os.path.isdir(_p) and _p not in sys.path:
        sys.path.append(_p)

_STATE: dict = {}
_OP2 = None

_OPS_CACHE: dict = {}


def _register_hand_op(name, make_uops, rd1, ref, spec_body):
    """Register a hand-written custom DVE op; returns the DveOp."""
    import concourse.dve_ops as dve_ops
    from concourse.dve_ops import DveOp, OPS
    from concourse.dve_spec import Spec
    from concourse.dve_uop import DveOpSpec

    for op in OPS:
        if op.name == name:
            return op

    class _H(DveOp):
        def compile(self, ver):
            spec = DveOpSpec(
                name=self.name,
                opcode=dve_ops.get_dve_sub_opcode(self.name),
                uops=make_uops(),
                rd1_en=rd1,
            )
            spec.validate(ver)
            return spec

    op = _H(name=name, spec=Spec(body=spec_body, reference=ref),
            subdim=False, uops_sha={})
    dve_ops._SUB_OPCODE_FOR_NAME[op.name] = 1 + len(OPS)
    OPS.append(op)
    dve_ops.CUSTOM_DVE_SPECS[op.name] = op.spec
    return op


def _register_spec_op(name, body, ref):
    """Register a Part-I Spec op (auto-lowered), bypassing the sha pin."""
    import concourse.dve_ops as dve_ops
    from concourse.dve_ops import DveOp, OPS
    from concourse.dve_spec import Spec, lower, _has_src1
    from concourse.dve_uop import DveOpSpec

    for op in OPS:
        if op.name == name:
            return op

    class _S(DveOp):
        def compile(self, ver):
            spec = DveOpSpec(
                name=self.name,
                opcode=dve_ops.get_dve_sub_opcode(self.name),
                uops=lower(self.spec, ver=ver),
                rd1_en=_has_src1(self.spec),
            )
            spec.validate(ver)
            return spec

    op = _S(name=name, spec=Spec(body=body, reference=ref),
            subdim=False, uops_sha={})
    dve_ops._SUB_OPCODE_FOR_NAME[op.name] = 1 + len(OPS)
    OPS.append(op)
    dve_ops.CUSTOM_DVE_SPECS[op.name] = op.spec
    return op


def get_poly_ops():
    """c1 = 1 + v*s0 + v^2*s1 ; gamma = -1 + v*s0 - v^2*s1 (= -c2/c1 +O(v^4))."""
    from concourse.dve_spec import Src0, C0, C1, One, sq

    c1 = _register_spec_op(
        "C1_POLY_ANT",
        Src0 * C0 + sq(Src0) * C1 + One,
        lambda in0, in1, s0, s1, imm2: (
            1.0 + in0.astype("float64") * s0 + in0.astype("float64") ** 2 * s1
        ).astype("float32"),
    )
    gm = _register_spec_op(
        "GAMMA_POLY_ANT",
        Src0 * C0 - sq(Src0) * C1 - One,
        lambda in0, in1, s0, s1, imm2: (
            in0.astype("float64") * s0 - in0.astype("float64") ** 2 * s1 - 1.0
        ).astype("float32"),
    )
    return c1, gm


def get_dbl_op():
    """Two PDE rows per instruction; see exp_custom3.py for the derivation.

    Stream (3 elements per fine column j): SRC_0 = [gamma_j, c1_j, c1_j],
    SRC_1 = K0[j+1] (x3). Per pair: u' = c1*(u + K0[j+1] + gamma*K0[j]),
    v' = c1*(v + u' + gamma*u); only v' is emitted (the intermediate row u
    never materializes). v is handed across elements via slice 7's a-flop
    (alu_out_a_enable on O, NEXT_ALU_OUT_A read on C at slice 6).
    """
    from concourse.dve_uop import (
        UopConfig, AluOp, AluInp, InpSel, OutSel, OutPath, Trigger, DelayInp,
    )
    import numpy as np

    def mk_init():
        u = UopConfig()
        u.enable_input(InpSel.ONE_F32, 1)
        u.repeat_count = 1
        u.trigger = (Trigger.COUNT, Trigger.NONE, Trigger.NONE)
        u.next_uop = (1, 0, 0)
        dp = u.datapath_config
        dp[0].enable_alu(AluOp.BYPASS, AluInp.PREV_DELAY_0)
        for b in range(7):
            dp[b].pass_through_delay(0)
        dp[5].enable_alu(AluOp.BYPASS, AluInp.PREV_DELAY_0)
        dp[7].enable_alu(AluOp.BYPASS, AluInp.PREV_DELAY_0)
        dp[7].alu_out_a_enable = 1
        return u

    def mk_O():
        u = UopConfig()
        u.enable_input(InpSel.SRC_0, 1)
        u.enable_input(InpSel.SRC_1, 2)
        u.require_inp0 = u.require_inp1 = 1
        u.repeat_count = 1
        u.trigger = (Trigger.SRC_TENSOR_DONE, Trigger.COUNT, Trigger.NONE)
        u.next_uop = (0, 2, 0)
        dp = u.datapath_config
        dp[0].enable_alu(AluOp.MULTIPLY, AluInp.PREV_DELAY_0, AluInp.CURR_ALU_OUT)
        dp[0].pass_through_delay(0, 1)
        dp[1].enable_alu(AluOp.ADD, AluInp.PREV_ALU_OUT, AluInp.PREV_DELAY_1)
        dp[1].pass_through_delay(0)
        dp[2].enable_delay_from_src(DelayInp.PREV_ALU_OUT, 1)
        dp[2].pass_through_delay(0)
        dp[3].enable_alu(AluOp.BYPASS, AluInp.PREV_DELAY_0)  # flop3 := gamma_j
        dp[3].pass_through_delay(0, 1)
        dp[4].pass_through_delay(0, 1)
        dp[5].enable_alu(AluOp.ADD, AluInp.PREV_DELAY_1, AluInp.CURR_ALU_OUT)
        dp[5].enable_delay_from_src(DelayInp.CURR_ALU_OUT, 2)
        dp[5].pass_through_delay(0)
        dp[6].enable_alu(AluOp.MULTIPLY, AluInp.PREV_DELAY_0, AluInp.PREV_DELAY_2)
        dp[7].enable_alu(AluOp.BYPASS, AluInp.CURR_ALU_OUT, AluInp.CURR_ALU_OUT)
        dp[7].alu_out_a_enable = 1
        return u

    def mk_E():
        u = UopConfig()
        u.enable_input(InpSel.SRC_0, 1)
        u.enable_input(InpSel.SRC_1, 2)
        u.require_inp0 = u.require_inp1 = 1
        u.repeat_count = 1
        u.trigger = (Trigger.SRC_TENSOR_DONE, Trigger.COUNT, Trigger.NONE)
        u.next_uop = (0, 3, 0)
        dp = u.datapath_config
        dp[0].enable_alu(AluOp.BYPASS, AluInp.PREV_DELAY_1)
        for b in range(3):
            dp[b].pass_through_delay(0)
        # s3: c1 = v - gamma (gamma parked in flop3 by O)
        dp[3].enable_alu(AluOp.SUBTRACT, AluInp.PREV_DELAY_0, AluInp.CURR_ALU_OUT)
        dp[4].pass_through_alu()  # carry c1
        dp[5].enable_alu(AluOp.MULTIPLY, AluInp.CURR_ALU_OUT, AluInp.PREV_ALU_OUT)
        dp[6].enable_alu(AluOp.ADD, AluInp.CURR_ALU_OUT, AluInp.PREV_ALU_OUT)
        return u

    def mk_C():
        u = UopConfig()
        u.enable_input(InpSel.SRC_0, 1)
        u.enable_input(InpSel.SRC_1, 2)
        u.require_inp0 = u.require_inp1 = 1
        u.repeat_count = 1
        u.trigger = (Trigger.SRC_TENSOR_DONE, Trigger.COUNT, Trigger.NONE)
        u.next_uop = (0, 1, 0)
        dp = u.datapath_config
        for b in range(6):
            dp[b].pass_through_delay(0)
        # s3: pick up c1 (flop3, written by E one cycle earlier) into lane1
        dp[3].enable_delay_from_src(DelayInp.CURR_ALU_OUT, 1)
        dp[4].pass_through_delay(1)
        dp[5].pass_through_delay(1)
        dp[6].enable_alu(AluOp.ADD, AluInp.CURR_ALU_OUT, AluInp.NEXT_ALU_OUT_A)
        dp[6].pass_through_delay(0, 1)
        dp[7].enable_alu(AluOp.MULTIPLY, AluInp.PREV_ALU_OUT, AluInp.PREV_DELAY_1)
        u.enable_output(OutSel.ALU_OUT, OutPath.WR0_LO)
        return u

    def ref(in0, in1, s0, s1, imm2):
        p = in0.shape[0]
        n = in0.shape[-1] // 3
        cc = in0.reshape(p, n, 3).astype(np.float64)
        kk = in1.reshape(p, n, 3)[:, :, 0].astype(np.float64)
        gam = cc[:, :, 0]
        c1 = cc[:, :, 1] - gam  # slot 1 carries v; c1 = v - gamma
        out = np.empty((p, n), np.float64)
        u = np.ones(p); v = np.ones(p); kprev = np.ones(p)
        for j in range(n):
            un = c1[:, j] * (u + kk[:, j] + gam[:, j] * kprev)
            v = c1[:, j] * (v + un + gam[:, j] * u)
            u, kprev = un, kk[:, j]
            out[:, j] = v
        return out.astype(np.float32)

    from concourse.dve_spec import Src0, Src1

    return _register_hand_op(
        "DBL_PAIR_SCAN_ANT", lambda: [mk_init(), mk_O(), mk_E(), mk_C()],
        True, ref, Src0 * Src1,
    )




def get_vf_scan_op():
    """Custom DVE op VF_PAIR_SCAN_ANT (hand-written 3-uop program).

    Reads the coarse vf value v_j (duplicated 4x via a stride-0 AP) and the
    previous K row (K[j+1] duplicated 2x), computes c1/gamma inline:
        c1 = 1 + v/2 + v^2/12,  gamma = -1 + v/2 - v^2/12  (= -c2/c1 + O(v^4))
        acc_j = c1_j * (acc_{j-1} + K[j+1] + gamma_j * K[j])
    and emits acc_j (= K_new[j+1]) on every second stream element, at one
    stream element per cycle. s0 = 0.5 (CONST_0), s1 = 1/12 (CONST_1).
    """
    global _OP2
    if _OP2 is not None:
        return _OP2
    import concourse.dve_ops as dve_ops
    from concourse.dve_ops import DveOp, OPS
    from concourse.dve_spec import Spec, Src0, Src1, C0, C1
    from concourse.dve_uop import (
        DveOpSpec,
        UopConfig,
        AluOp,
        AluInp,
        InpSel,
        OutSel,
        OutPath,
        Trigger,
        DelayInp,
    )

    for op in OPS:
        if op.name == "VF_PAIR_SCAN_ANT":
            _OP2 = op
            return op

    def _inputs(u):
        u.enable_input(InpSel.SRC_0, 0)    # v -> blk0 PREV_ALU_OUT
        u.enable_input(InpSel.SRC_0, 1)    # v -> lane 0
        u.enable_input(InpSel.SRC_1, 2)    # K[j+1] -> lane 1
        u.enable_input(InpSel.CONST_0, 3)  # 0.5 -> lane 2
        u.enable_input(InpSel.CONST_1, 4)  # 1/12 -> lane 3
        u.enable_input(InpSel.ONE_F32, 5)  # 1.0 -> lane 4
        u.require_inp0 = u.require_inp1 = 1
        u.repeat_count = 1
        return u

    def _mk_init():
        u = UopConfig()
        u.enable_input(InpSel.ONE_F32, 1)  # lane 0
        u.repeat_count = 1
        u.trigger = (Trigger.COUNT, Trigger.NONE, Trigger.NONE)
        u.next_uop = (1, 0, 0)
        dp = u.datapath_config
        dp[0].pass_through_delay(0)
        dp[1].enable_alu(AluOp.BYPASS, AluInp.PREV_DELAY_0)  # flop1 := 1.0 (K[0])
        for b in (1, 2, 3, 4, 5, 6):
            dp[b].pass_through_delay(0)
        dp[7].enable_alu(AluOp.BYPASS, AluInp.PREV_DELAY_0)  # flop7 := 1.0 (acc)
        return u

    def _mk_odd():
        # even stream elements (2j): compute gamma_j, p, m, u
        u = _inputs(UopConfig())
        u.trigger = (Trigger.SRC_TENSOR_DONE, Trigger.COUNT, Trigger.NONE)
        u.next_uop = (0, 2, 0)
        dp = u.datapath_config
        dp[0].enable_alu(AluOp.MULTIPLY, AluInp.PREV_ALU_OUT, AluInp.PREV_DELAY_2)
        dp[0].pass_through_delay(0, 1, 3, 4)
        dp[1].enable_alu(AluOp.MULTIPLY, AluInp.PREV_DELAY_0, AluInp.PREV_DELAY_0)
        dp[1].enable_delay_from_src(DelayInp.CURR_ALU_OUT, 5)  # K[j] handoff read
        dp[1].enable_delay_from_src(DelayInp.PREV_ALU_OUT, 0)  # a = v/2
        dp[1].pass_through_delay(1, 3, 4)
        dp[2].enable_alu(AluOp.MULTIPLY, AluInp.PREV_ALU_OUT, AluInp.PREV_DELAY_3)
        dp[2].pass_through_delay(0, 1, 4, 5)
        dp[3].enable_alu(AluOp.SUBTRACT, AluInp.PREV_DELAY_0, AluInp.PREV_DELAY_4)
        dp[3].enable_delay_from_src(DelayInp.PREV_ALU_OUT, 0)  # b = vv/12
        dp[3].pass_through_delay(1, 5)
        dp[4].enable_alu(AluOp.SUBTRACT, AluInp.PREV_ALU_OUT, AluInp.PREV_DELAY_0)
        dp[4].pass_through_delay(1, 5)
        dp[5].enable_alu(AluOp.MULTIPLY, AluInp.PREV_ALU_OUT, AluInp.PREV_DELAY_5)
        dp[5].pass_through_delay(1)
        dp[6].enable_alu(AluOp.ADD, AluInp.PREV_ALU_OUT, AluInp.PREV_DELAY_1)
        dp[7].enable_alu(AluOp.ADD, AluInp.PREV_ALU_OUT, AluInp.CURR_ALU_OUT)
        return u

    def _mk_even():
        # odd stream elements (2j+1): c1_j, K handoff, acc = u * c1; emit
        u = _inputs(UopConfig())
        u.trigger = (Trigger.SRC_TENSOR_DONE, Trigger.COUNT, Trigger.NONE)
        u.next_uop = (0, 1, 0)
        dp = u.datapath_config
        dp[0].enable_alu(AluOp.MULTIPLY, AluInp.PREV_ALU_OUT, AluInp.PREV_DELAY_0)
        dp[0].pass_through_delay(0, 1, 2, 3, 4)
        dp[1].enable_alu(AluOp.BYPASS, AluInp.PREV_DELAY_1)   # flop1 := K[j+1]
        dp[1].enable_delay_from_src(DelayInp.PREV_ALU_OUT, 5)  # vv
        dp[1].pass_through_delay(0, 2, 3, 4)
        dp[2].enable_alu(AluOp.MULTIPLY, AluInp.PREV_DELAY_0, AluInp.PREV_DELAY_2)
        dp[2].pass_through_delay(3, 4, 5)
        dp[3].enable_alu(AluOp.MULTIPLY, AluInp.PREV_DELAY_5, AluInp.PREV_DELAY_3)
        dp[3].enable_delay_from_src(DelayInp.PREV_ALU_OUT, 0)  # a = v/2
        dp[3].pass_through_delay(4)
        dp[4].enable_alu(AluOp.ADD, AluInp.PREV_DELAY_0, AluInp.PREV_DELAY_4)
        dp[4].enable_delay_from_src(DelayInp.PREV_ALU_OUT, 1)  # b = vv/12
        dp[5].enable_alu(AluOp.ADD, AluInp.PREV_ALU_OUT, AluInp.PREV_DELAY_1)
        dp[6].pass_through_alu()
        dp[7].enable_alu(AluOp.MULTIPLY, AluInp.CURR_ALU_OUT, AluInp.PREV_ALU_OUT)
        u.enable_output(OutSel.ALU_OUT, OutPath.WR0_LO)
        return u

    def _ref(in0, in1, s0, s1, imm2):
        p = in0.shape[0]
        n = in1.reshape(p, -1, 2).shape[1]
        v = in0.reshape(p, n, 2)[:, :, 0].astype(np.float64)
        kk = in1.reshape(p, n, 2)[:, :, 0].astype(np.float64)
        c1 = 1.0 + v * s0 + v * v * s1
        gam = -1.0 + v * s0 - v * v * s1
        out = np.empty((p, n), np.float64)
        acc = np.ones(p)
        kprev = np.ones(p)
        for j in range(n):
            acc = c1[:, j] * (acc + kk[:, j] + gam[:, j] * kprev)
            kprev = kk[:, j]
            out[:, j] = acc
        return out.astype(np.float32)

    class _HandDveOp2(DveOp):
        def compile(self, ver):
            spec = DveOpSpec(
                name=self.name,
                opcode=dve_ops.get_dve_sub_opcode(self.name),
                uops=[_mk_init(), _mk_odd(), _mk_even()],
                rd1_en=True,
            )
            spec.validate(ver)
            return spec

    op = _HandDveOp2(
        name="VF_PAIR_SCAN_ANT",
        spec=Spec(body=Src0 * Src1 * C0 * C1, reference=_ref),
        subdim=False,
        uops_sha={},
    )
    dve_ops._SUB_OPCODE_FOR_NAME[op.name] = 1 + len(OPS)
    OPS.append(op)
    dve_ops.CUSTOM_DVE_SPECS[op.name] = op.spec
    return get_vf_scan_op()

# (len, start) chunks of coarse rows J: producers run chunk-by-chunk so the
# PDE row loop can start after the first chunk.
JCH = [(1, 0), (1, 1), (2, 2), (3, 4), (4, 7), (6, 11), (8, 17), (8, 25),
       (8, 33), (8, 41), (8, 49), (6, 57)]


def _build_program():
    from contextlib import ExitStack

    import concourse.tile as tile
    from concourse import bacc, mybir

    DBL = get_dbl_op()
    f32 = mybir.dt.float32
    bf16 = mybir.dt.bfloat16
    Act = mybir.ActivationFunctionType

    nc = bacc.Bacc(
        "TRN2",
        target_bir_lowering=False,
        debug=False,
        enable_asserts=True,
        num_devices=8,
    )
    dyh_d = nc.dram_tensor("dyh", [32, 63 * 32], bf16, kind="ExternalInput").ap()
    dxh_d = nc.dram_tensor("dxh", [32, 4 * 63], bf16, kind="ExternalInput").ap()
    out_d = nc.dram_tensor("out", [4, 32], f32, kind="ExternalOutput").ap()

    with ExitStack() as ctx:
        tc = ctx.enter_context(tile.TileContext(nc))
        ws = ctx.enter_context(tc.tile_pool(name="ws", bufs=1))
        pp = ctx.enter_context(tc.tile_pool(name="pp", bufs=1, space="PSUM"))

        # Per-quadrant copies: group g's dy replica and dx slice live at
        # SBUF partitions 32g..32g+16 so the four matmul streams run as
        # independent diagonal PE tiles (tile_position (32g, 32g)).
        dxh_v = dxh_d.rearrange("d (g i) -> d g i", g=4)
        dyh_v = dyh_d.rearrange("d (j b) -> d j b", j=63)
        dxh_sb = ws.tile([128, 63], bf16)
        dyh_sb = ws.tile([128, 63, 32], bf16)
        # Distribute dma_start issue across engine queues: a DIRECT2D
        # issue costs ~600 ns on its sequencer, so 24 serial issues on Sync
        # would add ~14 us of head latency. Two waves: j 0:8 first so the
        # first matmul chunk's data lands fast.
        engs = [nc.sync, nc.scalar, nc.gpsimd]
        _ei = [0]

        def dma(out, in_):
            engs[_ei[0] % len(engs)].dma_start(out=out, in_=in_)
            _ei[0] += 1

        JW = 8
        for g in range(4):
            qs = slice(32 * g, 32 * g + 32)
            dma(dyh_sb[qs, 0:JW, :], dyh_v[:, 0:JW, :])
            dma(dxh_sb[qs, :], dxh_v[:, g, :])
        for g in range(4):
            qs = slice(32 * g, 32 * g + 32)
            dma(dyh_sb[qs, JW:63, :], dyh_v[:, JW:63, :])

        # K-row ping-pong buffers; 1.0 everywhere gives both the j=0 boundary
        # column and the i=0 boundary row (row 0 of K is all ones).
        kbuf = ws.tile([128, 2, 128], f32)
        nc.vector.memset(kbuf[:, :, :], 1.0)

        ps = pp.tile([128, 63, 64], f32)  # coarse inc; row J at [:, J, 0:63]
        # ccc: coarse interleaved [gamma_J, c1_J] pairs per row (stride-2
        # writes from the poly ops). cc3: fine stream [gamma_j, c1_j, c1_j]
        # per fine column (Act-expanded; the double-row op's SRC_0).
        ccc = ws.tile([128, 63, 63], f32)
        cc3 = ws.tile([128, 63, 63, 2, 3], f32)
        C1P, GMP = get_poly_ops()

        def produce(ln, st):
            jsl = slice(st, st + ln)
            for j in range(st, st + ln):
                for g in range(4):
                    po = slice(32 * g, 32 * g + 32)
                    qs = slice(32 * g, 32 * g + 32)
                    tp = (32 * g, 32 * g)
                    # K=32 contraction-concatenation: [dyh; dyh] x [dxh; dxl]
                    # = dyh*dxh + dyh*dxl in ONE matmul (half the ldweights).
                    nc.tensor.matmul(
                        ps[po, j, 0:63], dyh_sb[qs, j, :], dxh_sb[qs, :],
                        start=True, stop=True, tile_position=tp,
                    )
            nc.vector._custom_dve(
                GMP, out=ccc[:, jsl, :], in0=ps[:, jsl, 0:63],
                s0=0.5, s1=1.0 / 12.0, imm2=0.0,
            )
            # Act expansion to the fine [gamma, v, *] stream (the row op
            # derives c1 = v - gamma inline): gamma -> (J, d, 0), v ->
            # (J, d, 1). Slot 2 is never read by any ALU so it stays
            # unwritten. v reads PSUM directly (3-free-dim APs only).
            gsrc = ccc[:, jsl, :].unsqueeze(3).broadcast_to((128, ln, 63, 2))
            vsrc = ps[:, jsl, 0:63].unsqueeze(3).broadcast_to((128, ln, 63, 2))
            nc.scalar.activation(
                out=cc3[:, jsl, :, :, 0], in_=gsrc,
                func=Act.Copy, bias=0.0, scale=1.0,
            )
            nc.scalar.activation(
                out=cc3[:, jsl, :, :, 1], in_=vsrc,
                func=Act.Copy, bias=0.0, scale=1.0,
            )
        def rows(ln, st):
            for ri in range(st, st + ln):
                srcb = kbuf[:, ri & 1, :]
                dstb = kbuf[:, 1 - (ri & 1), :]
                win = srcb[:, 1:127].unsqueeze(2).broadcast_to((128, 126, 3))
                cin = cc3[:, ri, :, :, :].rearrange("p j d s -> p (j d s)")
                nc.vector._custom_dve(
                    DBL, out=dstb[:, 1:127], in0=cin, in1=win,
                    s0=0.0, s1=0.0, imm2=0.0,
                )

        # Software-pipelined issue with one-chunk lookahead: the DVE and Act
        # queues are in-order, so chunk c+1's producers are issued before
        # chunk c's rows; the Act expansion latency hides behind row
        # execution instead of stalling each chunk boundary.
        for ci in range(len(JCH) + 2):
            if ci < len(JCH):
                produce(*JCH[ci])
            if ci >= 2:
                rows(*JCH[ci - 2])

        # Compact the result column K[*, 126] (in kbuf[:, 0, :] after row 125)
        # into 4 partitions via a 32x32 block transpose, then one 4-descriptor
        # DMA: tt[32B + i, f] = kbuf[32B + f, 0, 96 + i] -> row i=30 holds col
        # 126 for the 32 pairs of partition-block B.
        # Compact result column K[*, 126] via 32x32 block transpose, then
        # four 1-descriptor DMAs (vs 128 4-byte descriptors, ~7 us).
        tt = ws.tile([128, 32], f32)
        nc.vector.transpose(tt[:], kbuf[:, 1, 96:128])
        oeng = [nc.scalar, nc.gpsimd, nc.sync, nc.scalar]
        for g in range(4):
            oeng[g].dma_start(
                out=out_d[g : g + 1, :], in_=tt[32 * g + 30 : 32 * g + 31, :]
            )

    nc.compile()
    return nc


def _get_nc():
    if "nc" not in _STATE:
        _STATE["nc"] = _build_program()
    return _STATE["nc"]


def _make_inputs(xs: np.ndarray, ys: np.ndarray):
    xs = np.asarray(xs, dtype=np.float32)
    ys = np.asarray(ys, dtype=np.float32)
    dxs_all = (xs[:, 1:, :] - xs[:, :-1, :]) * np.float32(0.25)  # (32, 63, 16)
    dys = ys[:, 1:, :] - ys[:, :-1, :]                           # (32, 63, 16)

    import ml_dtypes

    bf16 = ml_dtypes.bfloat16
    dysT = np.ascontiguousarray(dys.transpose(2, 1, 0))          # [d, j, b]
    dyh1 = dysT.astype(bf16).reshape(16, 63 * 32)
    dyh = np.ascontiguousarray(np.concatenate([dyh1, dyh1], axis=0))

    in_maps = []
    for c in range(8):
        dxs_c = np.ascontiguousarray(
            dxs_all[4 * c : 4 * c + 4].transpose(2, 0, 1).reshape(16, 4 * 63)
        )  # [d, (a', i)]
        dxhh = dxs_c.astype(bf16)
        dxll = (dxs_c - dxhh.astype(np.float32)).astype(bf16)
        dxh = np.ascontiguousarray(np.concatenate([dxhh, dxll], axis=0))
        in_maps.append({"dyh": dyh, "dxh": dxh})
    return in_maps


def _run(nc, in_maps, **kwargs):
    from concourse.bass_utils import run_bass_kernel_spmd

    return run_bass_kernel_spmd(nc, in_maps, list(range(8)), **kwargs)


def kernel(xs: np.ndarray, ys: np.ndarray) -> np.ndarray:
    nc = _get_nc()
    in_maps = _make_inputs(xs, ys)
    res = _run(nc, in_maps)
    out = np.concatenate(
        [np.asarray(res.results[c]["out"]).reshape(4, 32) for c in range(8)], axis=0
    )
    return out.astype(np.float32)
